# revision 1
# baseline (speedup 1.0000x reference)
"""Autoformer encoder (B=32, L=1024, D=256, 3 layers) on 8 TRN2 NeuronCores.

Data-parallel over batch (4 batches/core). All matmuls in f32r (full PE
rate, ~1.6e-4 rel err), fp32 residual stream and vector path.

AutoCorrelation without FFT: the lag-correlation
    C[tau] = (1/D) sum_l <q[:, l+tau], k[:, l]>
is computed as F[p, u] = sum_i sum_d k[d, 128i+p] * q2[d, 128i+u]
(PSUM-accumulated matmuls, q2 time-doubled), so that
C[tau] = sum_p F[p, p+tau]. The 128-row shear is done by bouncing F
through DRAM with row stride 1153 and reading back the strided view
[[1154, 128], [1, 1024]]; the partition sum is a ones-vector matmul.
Top-6 lags via vector.max/max_index.

The delay-rolled weighted sum of V uses register-dynamic slices into a
time-doubled V buffer. Each dynamic-AP instruction permanently consumes
~2 registers of the executing engine's 49 (no caching in this build), so
the 72 total gather slots are spread: 23 on ACT (scaled copy), 23 on DVE
(scalar_tensor_tensor FMA), 22 on Pool (FMA), 4 on PE (scaled-identity
matmul with dynamic rhs, PSUM-accumulated).
"""

import contextlib
import numpy as np
import ml_dtypes

import concourse.bass as bass
import concourse.mybir as mybir
from concourse import tile
from concourse.tile import TileContext
from concourse.tile_rust import add_dep_helper
from concourse.vector_clock import ScopedClock
from concourse.bass_utils import run_bass_kernel_spmd

F32 = mybir.dt.float32
F32R = mybir.dt.float32r
BF16 = mybir.dt.bfloat16
U32 = mybir.dt.uint32
AF = mybir.ActivationFunctionType
AX = mybir.AxisListType
ALU = mybir.AluOpType
ET = mybir.EngineType

B, L, C_IN = 32, 1024, 21
D, DFF, NL = 256, 1024, 3
TOPK = 6
NCORES = 8
BL = B // NCORES  # batches per core

HW = 1153  # F bounce row stride (1152 data + 1 pad)
FSH_SZ = 127 * HW + 1152


# ---------------------------------------------------------------- walrus fix
def _patched_drain_and_barrier(self, tick_clock, wait_clock):
    nc = self.nc
    drain_inst = nc.sync.drain()
    wait_clock.add_sem_waits(
        drain_inst.ins, ScopedClock({None: tick_clock.global_clock})
    )
    si = drain_inst.ins.sync_info
    if si is not None and len(si.on_wait) > 1:
        extra = list(si.on_wait[1:])
        del si.on_wait[1:]
        for w in extra:
            n = nc.sync.nop()
            n.ins.sync_info = mybir.SyncInfo(on_update=[], on_wait=[w])
    nc.all_engine_barrier()
    assert self.sems is not None
    popped = nc._tile_sem_poison_stack.pop()
    assert popped is self._sem_poison
    nc.clear_and_free_semaphores(list(self.sems.allocated().values()))
    nc.all_engine_barrier()


tile.TileContext._drain_and_barrier = _patched_drain_and_barrier

_wsctr = [0]


def _split_control_waits(nc):
    """This walrus build allows only ONE sync wait per instruction;
    hoist extras onto NoOps just before, same engine."""
    for fn in nc.m.functions:
        for bb in fn.blocks:
            out = []
            changed = False
            for inst in bb.instructions:
                si = getattr(inst, "sync_info", None)
                if si is not None and len(si.on_wait) > 1:
                    extra = list(si.on_wait[1:])
                    del si.on_wait[1:]
                    for w in extra:
                        _wsctr[0] += 1
                        n = mybir.InstNoOp(
                            name=f"I-waitsplit-{_wsctr[0]}", ins=[], outs=[]
                        )
                        n.engine = inst.engine
                        n.sync_info = mybir.SyncInfo(on_update=[], on_wait=[w])
                        out.append(n)
                        changed = True
                out.append(inst)
            if changed:
                bb.instructions[:] = out


def r(ap):
    return ap


def dep(a, b):
    add_dep_helper(a.ins, b.ins, sync=False, reason="gather order")


# ---------------------------------------------------------------- builder
def build_nc():
    nc = bass.Bass()
    P = lambda name, shape, dt=F32: nc.declare_dram_parameter(
        name, shape, dt, isOutput=False
    )
    xemb = P("xemb", [BL, 63, L], F32R)  # host im2col of token conv input
    tokw = P("tokw", [63, D], F32R)  # lhsT for token conv
    wq = P("wq", [NL, D, D], F32R)  # lhsT (= W.T) per layer
    wk = P("wk", [NL, D, D], F32R)
    wv = P("wv", [NL, D, D], F32R)
    wo = P("wo", [NL, D, D], F32R)
    wc1 = P("wc1", [NL, D, DFF], F32R)  # lhsT
    wc2 = P("wc2", [NL, DFF, D], F32R)  # lhsT
    nwp = P("nw", [D, 1])
    nbp = P("nb", [D, 1])
    pw = P("pw", [D, 3, L], BF16)  # proj_w as [d, class, l], bf16
    pb = P("pb", [1, 3])
    onescol = P("onescol", [128, 1], F32R)
    onescolf = P("onescolf", [128, 1])
    onesrow = P("onesrow", [1, 128])
    ident = P("ident", [128, 128])
    out = nc.declare_dram_parameter("out", [BL, 3], F32, isOutput=True)
    import os
    KDBG = bool(os.environ.get("KDBG"))
    dbg = {}
    if KDBG:
        for nm_, shp, dt_ in [
            ("dbg_x0", [2, 128, L], F32),
            ("dbg_k", [128, L], F32),
            ("dbg_q2", [128, 4096], F32),
            ("dbg_f", [128, 1152], F32),
            ("dbg_h", [128, L], F32),
            ("dbg_c", [1, L], F32),
            ("dbg_ix", [1, 8], U32),
            ("dbg_wb", [128, 8], F32),
            ("dbg_a", [128, 2048], F32),
            ("dbg_s", [2, 128, L + 4], F32),
            ("dbg_x1", [2, 128, L + 4], F32),
            ("dbg_xo", [2, 128, L], F32),
        ]:
            dbg[nm_] = nc.declare_dram_parameter(nm_, shp, dt_, isOutput=True)

    fsh = nc.dram_tensor("fsh", [BL * NL, FSH_SZ], F32R)

    with TileContext(nc) as tc:
        ctx = contextlib.ExitStack()
        with ctx:
            wp = ctx.enter_context(tc.tile_pool(name="weights", bufs=1))
            res = ctx.enter_context(tc.tile_pool(name="res", bufs=6))
            scr = ctx.enter_context(tc.tile_pool(name="scr", bufs=8))
            big = ctx.enter_context(tc.tile_pool(name="big4k", bufs=3))
            gat = ctx.enter_context(tc.tile_pool(name="gat", bufs=3))
            sp = ctx.enter_context(tc.tile_pool(name="small", bufs=4))
            ps = ctx.enter_context(tc.tile_pool(name="psum", bufs=3, space="PSUM"))
            ps2p = ctx.enter_context(
                tc.tile_pool(name="psumB", bufs=2, space="PSUM")
            )

            _names = [0]

            def _nm(pfx):
                _names[0] += 1
                return f"{pfx}{_names[0]}"

            def rtile():
                return res.tile([128, L + 4], F32R, tag="res", name=_nm("rt"))

            def stile(fr=1152, dt=F32, p=128):
                return scr.tile([p, fr], dt, tag="scr", name=_nm("st"))

            # ---- load weights to SBUF once
            tokw_sb = wp.tile([63, D], F32R, tag="tokw")
            nc.sync.dma_start(out=tokw_sb[:], in_=tokw[:])
            ones_sb = wp.tile([128, 1], F32R, tag="ones")
            ones2_sb = wp.tile([128, 1], F32, tag="ones2")
            nc.sync.dma_start(out=ones_sb[:], in_=onescol[:])
            nc.sync.dma_start(out=ones2_sb[:], in_=onescolf[:])
            onesr_sb = wp.tile([1, 128], F32, tag="onesr")
            nc.sync.dma_start(out=onesr_sb[:], in_=onesrow[:])
            id_sb = wp.tile([128, 128], F32, tag="id")
            nc.sync.dma_start(out=id_sb[:], in_=ident[:])
            nw_sb = wp.tile([128, 2], F32, tag="nw")  # col t = tile t
            nb_sb = wp.tile([128, 2], F32, tag="nb")
            for t in range(2):
                nc.sync.dma_start(
                    out=nw_sb[:, t : t + 1], in_=nwp[t * 128 : (t + 1) * 128, :]
                )
                nc.sync.dma_start(
                    out=nb_sb[:, t : t + 1], in_=nbp[t * 128 : (t + 1) * 128, :]
                )
            pb_sb = wp.tile([1, 3], F32, tag="pb")
            nc.sync.dma_start(out=pb_sb[:], in_=pb[:])

            # layer weights streamed per (b, l), double-buffered
            ws = ctx.enter_context(tc.tile_pool(name="wstream", bufs=2))

            def lload(name, src, l, kt, cols):
                tl = ws.tile(
                    [128, cols], F32R, tag=f"{name}k{kt}", name=_nm(f"{name}{l}")
                )
                nc.sync.dma_start(out=tl[:], in_=src[l, kt * 128 : (kt + 1) * 128, :])
                return tl
            pw_sb = [None, None]
            for t in range(2):
                pw_sb[t] = wp.tile([128, 3 * L], BF16, tag=f"pw{t}", name=f"pw{t}")
                nc.sync.dma_start(
                    out=pw_sb[t][:].rearrange("p (c l) -> p c l", c=3),
                    in_=pw[t * 128 : (t + 1) * 128, :, :],
                )

            # persistent per-engine delay registers + snapped values
            engs = {
                "ACT": nc.engines[ET.Activation],
                "DVE": nc.engines[ET.DVE],
                "POOL": nc.engines[ET.Pool],
                "PE": nc.engines[ET.PE],
            }
            dreg = {k: e.alloc_register(f"dly_{k}") for k, e in engs.items()}
            dval = {
                k: nc.snap(rg, donate=True, min_val=0, max_val=1023)
                for k, rg in dreg.items()
            }

            def proj(dst_fn, w_sb_l, src_aps):
                """dst[mt][chunk] <- sum_kt w[kt].T @ src[kt][:, chunk]."""
                for mt in range(2):
                    for ch in range(2):
                        p5 = ps2p.tile([128, 512], F32, tag="mm512", name=_nm("p5"))
                        for kt in range(2):
                            nc.tensor.matmul(
                                p5[:],
                                r(w_sb_l[kt][:, mt * 128 : (mt + 1) * 128]),
                                r(src_aps[kt][:, ch * 512 : (ch + 1) * 512]),
                                start=(kt == 0),
                                stop=(kt == 1),
                            )
                        dst_fn(mt, ch, p5)

            def batch_program(b):
                # ---- token embedding: x[d, l], 2 tiles, data in [0, L)
                xe_sb = stile(fr=L, p=63, dt=F32R)
                nc.sync.dma_start(out=xe_sb[:], in_=xemb[b, :, :])
                x_sb = [rtile() for _ in range(2)]
                for mt in range(2):
                    for ch in range(2):
                        p5 = ps2p.tile([128, 512], F32, tag="mm512", name=_nm("p5"))
                        nc.tensor.matmul(
                            p5[:],
                            r(tokw_sb[:, mt * 128 : (mt + 1) * 128]),
                            r(xe_sb[:, ch * 512 : (ch + 1) * 512]),
                            start=True,
                            stop=True,
                        )
                        nc.vector.tensor_copy(
                            x_sb[mt][:, ch * 512 : (ch + 1) * 512], p5[:]
                        )

                if KDBG and b == 0:
                    for t in range(2):
                        nc.sync.dma_start(
                            out=dbg["dbg_x0"][t], in_=x_sb[t][:, 0:L].bitcast(F32)
                        )

                for l in range(NL):
                    last_bl = (b == BL - 1) and (l == NL - 1)
                    tap = KDBG and b == 0 and l == 0
                    wq_l = [lload("wq", wq, l, t, D) for t in range(2)]
                    wk_l = [lload("wk", wk, l, t, D) for t in range(2)]
                    wv_l = [lload("wv", wv, l, t, D) for t in range(2)]
                    wo_l = [lload("wo", wo, l, t, D) for t in range(2)]
                    wc1_l = [lload("wc1", wc1, l, t, DFF) for t in range(2)]
                    wc2_l = [lload("wc2", wc2, l, t, D) for t in range(8)]
                    # ---- Q (doubled, stacked kt: col 2048*kt + u), K, V (same)
                    q2_sb = big.tile([128, 4096], F32R, tag="big4k", name=_nm("q2"))
                    v4_sb = big.tile([128, 4096], F32R, tag="big4k", name=_nm("v4"))
                    k_sb = [stile(dt=F32R) for _ in range(2)]

                    def dbl_out(dst):
                        def f(mt, ch, p5):
                            base = 2048 * mt + ch * 512
                            nc.vector.tensor_copy(dst[:, base : base + 512], p5[:])
                            nc.scalar.copy(dst[:, base + 1024 : base + 1536], p5[:])

                        return f

                    def k_out(mt, ch, p5):
                        nc.scalar.copy(
                            k_sb[mt][:, ch * 512 : (ch + 1) * 512], p5[:]
                        )

                    xin = [x_sb[t][:, 0:L] for t in range(2)]
                    proj(dbl_out(q2_sb), wq_l, xin)
                    proj(k_out, wk_l, xin)
                    proj(dbl_out(v4_sb), wv_l, xin)

                    if tap:
                        nc.sync.dma_start(
                            out=dbg["dbg_k"][:], in_=k_sb[0][:, 0:L].bitcast(F32)
                        )
                        nc.sync.dma_start(
                            out=dbg["dbg_q2"][:], in_=q2_sb[:].bitcast(F32)
                        )

                    # ---- F[p, u] = sum_i sum_d k[d,128i+p] q2[d,128i+u]
                    # F in two PSUM tiles so "big" slots stay 2 banks and F
                    # can overlap the FFN's ps2 accumulators. Each 384-wide
                    # chunk is bank-aligned (a matmul output may not cross a
                    # 512-f32 PSUM bank).
                    fps_a = ps.tile([128, 1024], F32, tag="big", name=_nm("fpsa"))
                    fps_b = ps2p.tile([128, 512], F32, tag="mm512", name=_nm("fpsb"))
                    for ch in range(3):  # 3 x 384
                        dstp = (
                            fps_a[:, ch * 512 : ch * 512 + 384]
                            if ch < 2
                            else fps_b[:, 0:384]
                        )
                        for i in range(8):
                            for kt in range(2):
                                base = 2048 * kt + i * 128 + ch * 384
                                nc.tensor.matmul(
                                    dstp,
                                    r(k_sb[kt][:, i * 128 : (i + 1) * 128]),
                                    r(q2_sb[:, base : base + 384]),
                                    start=((i, kt) == (0, 0)),
                                    stop=((i, kt) == (7, 1)),
                                )
                    # bounce through DRAM with the shear stride
                    f_sb = stile(dt=F32R)
                    nc.vector.tensor_copy(
                        f_sb[:, 0:768].rearrange("p (c u) -> p c u", c=2),
                        fps_a[:].rearrange("p (c u) -> p c u", c=2)[:, :, 0:384],
                    )
                    nc.vector.tensor_copy(f_sb[:, 768:1152], fps_b[:, 0:384])
                    frow = fsh[b * NL + l, :]
                    wview = bass.AP(frow.tensor, frow.offset, [[HW, 128], [1, 1152]])
                    fwr = nc.sync.dma_start(out=wview, in_=f_sb[:, 0:1152])
                    hview = bass.AP(
                        frow.tensor, frow.offset, [[HW + 1, 128], [1, 1024]]
                    )
                    h_sb = stile(dt=F32R)
                    hrd = nc.sync.dma_start(out=h_sb[:, 0:1024], in_=hview)
                    add_dep_helper(
                        hrd.ins, fwr.ins, sync=True, reason="hankel read after write"
                    )
                    yield
                    if tap:
                        nc.sync.dma_start(
                            out=dbg["dbg_f"][:], in_=f_sb[:, 0:1152].bitcast(F32)
                        )
                        nc.sync.dma_start(
                            out=dbg["dbg_h"][:], in_=h_sb[:, 0:1024].bitcast(F32)
                        )

                    # ---- C[tau] = (1/256) * sum_p H[p, tau]; top-6; softmax
                    c_sb = stile()
                    for ch in range(2):
                        cp = ps2p.tile([1, 512], F32, tag="mm512", name=_nm("cp"))
                        nc.tensor.matmul(
                            cp[:],
                            r(ones_sb[:]),
                            r(h_sb[:, ch * 512 : (ch + 1) * 512]),
                            start=True,
                            stop=True,
                        )
                        nc.scalar.activation(
                            c_sb[:1, ch * 512 : (ch + 1) * 512],
                            cp[:],
                            AF.Copy,
                            scale=1.0 / D,
                        )
                    mx = sp.tile([1, 8], F32, tag="mx", name=_nm("mx"))
                    ix = sp.tile([1, 8], U32, tag="ix", name=_nm("ix"))
                    nc.vector.max(out=mx[:], in_=c_sb[:1, 0:1024])
                    nc.vector.max_index(
                        out=ix[:], in_max=mx[:], in_values=c_sb[:1, 0:1024]
                    )
                    negmax = sp.tile([1, 1], F32, tag="negmax", name=_nm("ng"))
                    nc.vector.tensor_scalar_mul(negmax[:], mx[:1, 0:1], -1.0)
                    ex = sp.tile([1, 8], F32, tag="ex", name=_nm("ex"))
                    nc.scalar.activation(
                        ex[:1, 0:TOPK], mx[:1, 0:TOPK], AF.Exp, bias=negmax[:1, 0:1]
                    )
                    esum = sp.tile([1, 1], F32, tag="esum", name=_nm("es"))
                    nc.vector.reduce_sum(esum[:], ex[:1, 0:TOPK], axis=AX.X)
                    rinv = sp.tile([1, 1], F32, tag="rinv", name=_nm("ri"))
                    nc.vector.reciprocal(rinv[:], esum[:])
                    wts = sp.tile([1, 8], F32, tag="wts", name=_nm("wt"))
                    nc.vector.tensor_scalar_mul(
                        wts[:1, 0:TOPK], ex[:1, 0:TOPK], rinv[:1, 0:1]
                    )
                    # broadcast weights to all 128 partitions
                    psw = ps2p.tile([128, TOPK], F32, tag="mm512", name=_nm("pw_"))
                    nc.tensor.matmul(
                        psw[:], onesr_sb[:], wts[:1, 0:TOPK], start=True, stop=True
                    )
                    wb = sp.tile([128, TOPK], F32, tag="wb", name=_nm("wb"))
                    nc.vector.tensor_copy(wb[:], psw[:])
                    if tap:
                        nc.sync.dma_start(out=dbg["dbg_c"][:], in_=c_sb[:1, 0:L])
                        nc.sync.dma_start(out=dbg["dbg_ix"][:], in_=ix[:])
                        nc.sync.dma_start(
                            out=dbg["dbg_wb"][:, 0:TOPK], in_=wb[:]
                        )

                    # ---- a[:, 1024*t + u] = sum_i w_i V[t][:, (u+d_i) % L]
                    a_sb = gat.tile([128, 2048], F32R, tag="gat", name=_nm("a"))
                    tq_sb = gat.tile([128, 2048], F32R, tag="gat", name=_nm("tq"))
                    pq_sb = gat.tile([128, 2048], F32R, tag="gat", name=_nm("pq"))
                    v4r = v4_sb[:].rearrange("p (b u) -> p b u", b=2)
                    a3 = a_sb[:].rearrange("p (b u) -> p b u", b=2)
                    tq3 = tq_sb[:].rearrange("p (b u) -> p b u", b=2)
                    pq3 = pq_sb[:].rearrange("p (b u) -> p b u", b=2)

                    def ld(ekey, i):
                        return engs[ekey].reg_load(dreg[ekey], ix[:1, i : i + 1])

                    def act_copy(i, dst3):
                        return nc.scalar.activation(
                            dst3,
                            v4r[:, :, bass.ds(dval["ACT"], 1024)],
                            AF.Copy,
                            scale=wb[:, i : i + 1],
                        )

                    def fma(ekey, i):
                        eng = nc.vector if ekey == "DVE" else nc.gpsimd
                        return eng.scalar_tensor_tensor(
                            a3,
                            v4r[:, :, bass.ds(dval[ekey], 1024)],
                            wb[:, i : i + 1],
                            a3,
                            op0=ALU.mult,
                            op1=ALU.add,
                        )

                    if not last_bl:
                        l0 = ld("ACT", 0)
                        o0 = act_copy(0, a3)
                        dep(o0, l0)
                        l1 = ld("ACT", 1)
                        dep(l1, o0)
                        o1 = act_copy(1, tq3)
                        dep(o1, l1)
                        l2 = ld("DVE", 2)
                        o2 = fma("DVE", 2)
                        dep(o2, l2)
                        l3 = ld("DVE", 3)
                        dep(l3, o2)
                        o3_ = fma("DVE", 3)
                        dep(o3_, l3)
                        # Pool: tensor_tensor mult with broadcast weight
                        l4 = ld("POOL", 4)
                        o4 = nc.gpsimd.tensor_mul(
                            pq3,
                            v4r[:, :, bass.ds(dval["POOL"], 1024)],
                            wb[:, 4:5].to_broadcast([128, 2, 1024]),
                        )
                        dep(o4, l4)
                        ad4 = nc.vector.tensor_add(a_sb[:], a_sb[:], pq_sb[:])
                        l5 = ld("POOL", 5)
                        dep(l5, o4)
                        o5 = nc.gpsimd.tensor_mul(
                            pq3,
                            v4r[:, :, bass.ds(dval["POOL"], 1024)],
                            wb[:, 5:6].to_broadcast([128, 2, 1024]),
                        )
                        dep(o5, l5)
                        nc.vector.tensor_add(a_sb[:], a_sb[:], pq_sb[:])
                        nc.vector.tensor_add(a_sb[:], a_sb[:], tq_sb[:])
                    else:
                        # last (b, l): ACT slot 0, DVE slot 1, PE slots 2..5
                        l0 = ld("ACT", 0)
                        o0 = act_copy(0, a3)
                        dep(o0, l0)
                        l1 = ld("DVE", 1)
                        o1 = fma("DVE", 1)
                        dep(o1, l1)
                        pe = engs["PE"]
                        wds = []
                        for i in range(2, 6):
                            wd = stile(fr=128, dt=F32R)
                            nc.vector.tensor_scalar(
                                wd[:, 0:128],
                                id_sb[:],
                                wb[:, i : i + 1],
                                None,
                                op0=ALU.mult,
                            )
                            wds.append(wd)
                        pgs = []
                        prev = None
                        for t in range(2):
                            for c in range(2):
                                pg = ps2p.tile(
                                    [128, 512], F32, tag="mm512", name=_nm("pg")
                                )
                                for ii, i in enumerate(range(2, 6)):
                                    lp = pe.reg_load(dreg["PE"], ix[:1, i : i + 1])
                                    if prev is not None:
                                        dep(lp, prev)
                                    al = pe.reg_alu(
                                        dreg["PE"],
                                        dreg["PE"],
                                        2048 * t + 512 * c,
                                        ALU.add,
                                    )
                                    dep(al, lp)
                                    mm = nc.tensor.matmul(
                                        pg[:],
                                        r(wds[ii][:, 0:128]),
                                        r(v4_sb[:, bass.ds(dval["PE"], 512)]),
                                        start=(ii == 0),
                                        stop=(ii == 3),
                                    )
                                    dep(mm, al)
                                    prev = mm
                                pgs.append((t, c, pg))
                        for t, c, pg in pgs:
                            base = 1024 * t + 512 * c
                            nc.vector.tensor_add(
                                a_sb[:, base : base + 512],
                                a_sb[:, base : base + 512],
                                pg[:],
                            )

                    if tap:
                        nc.sync.dma_start(
                            out=dbg["dbg_a"][:], in_=a_sb[:].bitcast(F32)
                        )

                    yield

                    # ---- O-projection; s = x + a into padded tile (data at 2)
                    s_sb = [rtile() for _ in range(2)]

                    def o_out(mt, ch, p5):
                        nc.vector.tensor_add(
                            s_sb[mt][:, 2 + ch * 512 : 2 + (ch + 1) * 512],
                            x_sb[mt][:, ch * 512 : (ch + 1) * 512],
                            p5[:],
                        )

                    proj(
                        o_out,
                        wo_l,
                        [a_sb[:, 1024 * t : 1024 * (t + 1)] for t in range(2)],
                    )

                    # ---- series_decomp (dst may alias src data cols)
                    def decomp(src_pad, dst, dst_off):
                        # src_pad: [128, 1028] with data in cols [2, 1026)
                        sv = src_pad[:, 2:1026]
                        nc.vector.tensor_copy(
                            src_pad[:, 0:2], src_pad[:, 2:3].to_broadcast([128, 2])
                        )
                        nc.vector.tensor_copy(
                            src_pad[:, 1026:1028],
                            src_pad[:, 1025:1026].to_broadcast([128, 2]),
                        )
                        a2 = stile()
                        nc.gpsimd.tensor_add(
                            a2[:, 0:1027], src_pad[:, 0:1027], src_pad[:, 1:1028]
                        )
                        a4 = stile()
                        nc.vector.tensor_add(
                            a4[:, 0:1025], a2[:, 0:1025], a2[:, 2:1027]
                        )
                        m5 = stile()
                        nc.vector.tensor_add(
                            m5[:, 0:1024], a4[:, 0:1024], src_pad[:, 4:1028]
                        )
                        # dst = (m5 * -0.2) + sv, fused
                        nc.vector.scalar_tensor_tensor(
                            dst[:, dst_off : dst_off + 1024],
                            m5[:, 0:1024],
                            -0.2,
                            sv,
                            op0=ALU.mult,
                            op1=ALU.add,
                        )

                    if tap:
                        for t in range(2):
                            nc.sync.dma_start(
                                out=dbg["dbg_s"][t], in_=s_sb[t][:].bitcast(F32)
                            )

                    # x1 = decomp(s) in place (x1 aliases s_sb data cols)
                    for t in range(2):
                        decomp(s_sb[t], s_sb[t], dst_off=2)
                    x1_sb = s_sb
                    if tap:
                        for t in range(2):
                            nc.sync.dma_start(
                                out=dbg["dbg_x1"][t], in_=x1_sb[t][:].bitcast(F32)
                            )

                    # ---- FFN: y = gelu(c1 @ x1); s2 = x1 + c2 @ y (in place)
                    x1v = [x1_sb[t][:, 2:1026] for t in range(2)]
                    ps2 = [
                        ps.tile([128, 1024], F32, tag="big", name=_nm("ps2"))
                        for _ in range(2)
                    ]
                    for ft in range(8):
                        y_sb = stile(dt=F32R)
                        for ch in range(2):
                            p5 = ps2p.tile([128, 512], F32, tag="mm512", name=_nm("p5"))
                            for kt in range(2):
                                nc.tensor.matmul(
                                    p5[:],
                                    r(wc1_l[kt][:, ft * 128 : (ft + 1) * 128]),
                                    r(x1v[kt][:, ch * 512 : (ch + 1) * 512]),
                                    start=(kt == 0),
                                    stop=(kt == 1),
                                )
                            nc.scalar.activation(
                                y_sb[:, ch * 512 : (ch + 1) * 512], p5[:], AF.Gelu
                            )
                        for mt in range(2):
                            for ch in range(2):
                                nc.tensor.matmul(
                                    ps2[mt][:, ch * 512 : (ch + 1) * 512],
                                    r(wc2_l[ft][:, mt * 128 : (mt + 1) * 128]),
                                    r(y_sb[:, ch * 512 : (ch + 1) * 512]),
                                    start=(ft == 0),
                                    stop=(ft == 7),
                                )
                    for mt in range(2):
                        for ch in range(2):
                            nc.vector.tensor_add(
                                x1v[mt][:, ch * 512 : (ch + 1) * 512],
                                x1v[mt][:, ch * 512 : (ch + 1) * 512],
                                ps2[mt][:, ch * 512 : (ch + 1) * 512],
                            )
                    for t in range(2):
                        decomp(x1_sb[t], x_sb[t], dst_off=0)
                    yield
                    if tap:
                        for t in range(2):
                            nc.sync.dma_start(
                                out=dbg["dbg_xo"][t], in_=x_sb[t][:, 0:L].bitcast(F32)
                            )

                # ---- my_layernorm + gelu + head
                xv = [x_sb[t][:, 0:L] for t in range(2)]
                xsq = [stile(dt=F32R) for _ in range(2)]
                for t in range(2):
                    nc.scalar.activation(xsq[t][:, 0:L], xv[t], AF.Square)
                mu = stile()
                ex2 = stile()
                for ch in range(2):
                    cs = ps2p.tile([1, 512], F32, tag="mm512", name=_nm("cs"))
                    for kt in range(2):
                        nc.tensor.matmul(
                            cs[:],
                            r(ones_sb[:]),
                            r(xv[kt][:, ch * 512 : (ch + 1) * 512]),
                            start=(kt == 0),
                            stop=(kt == 1),
                        )
                    nc.scalar.activation(
                        mu[:1, ch * 512 : (ch + 1) * 512], cs[:], AF.Copy, scale=1.0 / D
                    )
                    cq = ps2p.tile([1, 512], F32, tag="mm512", name=_nm("cq"))
                    for kt in range(2):
                        nc.tensor.matmul(
                            cq[:],
                            r(ones_sb[:]),
                            r(xsq[kt][:, ch * 512 : (ch + 1) * 512]),
                            start=(kt == 0),
                            stop=(kt == 1),
                        )
                    nc.scalar.activation(
                        ex2[:1, ch * 512 : (ch + 1) * 512],
                        cq[:],
                        AF.Copy,
                        scale=1.0 / D,
                    )
                epsb = sp.tile([1, 1], F32, tag="epsb", name=_nm("ep"))
                nc.vector.memset(epsb[:], 1e-5)
                musq = stile()
                nc.vector.tensor_mul(musq[:1, 0:1024], mu[:1, 0:1024], mu[:1, 0:1024])
                nc.vector.tensor_sub(
                    ex2[:1, 0:1024], ex2[:1, 0:1024], musq[:1, 0:1024]
                )
                nc.scalar.activation(
                    ex2[:1, 0:1024], ex2[:1, 0:1024], AF.Sqrt, bias=epsb[:1, 0:1]
                )
                nc.vector.reciprocal(ex2[:1, 0:1024], ex2[:1, 0:1024])  # rstd
                # broadcast mu, rstd to 128 partitions
                mub = stile()
                rstdb = stile()
                for src, dst in ((mu, mub), (ex2, rstdb)):
                    for ch in range(2):
                        pbd = ps2p.tile([128, 512], F32, tag="mm512", name=_nm("pb_"))
                        nc.tensor.matmul(
                            pbd[:],
                            onesr_sb[:],
                            src[:1, ch * 512 : (ch + 1) * 512],
                            start=True,
                            stop=True,
                        )
                        nc.vector.tensor_copy(dst[:, ch * 512 : (ch + 1) * 512], pbd[:])
                g_sb = [stile(dt=BF16) for _ in range(2)]
                for t in range(2):
                    xh = stile()
                    nc.vector.tensor_sub(xh[:, 0:L], xv[t], mub[:, 0:L])
                    nc.vector.tensor_mul(xh[:, 0:L], xh[:, 0:L], rstdb[:, 0:L])
                    nc.scalar.activation(
                        xh[:, 0:L],
                        xh[:, 0:L],
                        AF.Identity,
                        bias=nb_sb[:, t : t + 1],
                        scale=nw_sb[:, t : t + 1],
                    )
                    rowm = sp.tile([128, 1], F32, tag="rowm", name=_nm("rm"))
                    nc.vector.reduce_sum(rowm[:], xh[:, 0:L], axis=AX.X)
                    nc.vector.tensor_scalar_mul(rowm[:], rowm[:], 1.0 / L)
                    nc.vector.tensor_scalar_sub(xh[:, 0:L], xh[:, 0:L], rowm[:, 0:1])
                    nc.scalar.activation(g_sb[t][:, 0:L], xh[:, 0:L], AF.Gelu)

                # head: out[c] = sum_{t,p,l} g[t][p,l] * pw[t][p, c, l] + pb
                hsum = sp.tile([128, 8], F32, tag="hsum", name=_nm("hs"))
                for t in range(2):
                    for c in range(3):
                        hscr = stile()
                        nc.vector.tensor_mul(
                            hscr[:, 0:L],
                            g_sb[t][:, 0:L],
                            pw_sb[t][:, c * L : (c + 1) * L],
                        )
                        nc.vector.reduce_sum(
                            hsum[:, t * 3 + c : t * 3 + c + 1],
                            hscr[:, 0:L],
                            axis=AX.X,
                        )
                psh = ps2p.tile([1, 6], F32, tag="mm512", name=_nm("ph"))
                nc.tensor.matmul(
                    psh[:], ones2_sb[:], hsum[:, 0:6], start=True, stop=True
                )
                h6 = sp.tile([1, 6], F32, tag="h6", name=_nm("h6"))
                nc.vector.tensor_copy(h6[:], psh[:1, 0:6])
                o3 = sp.tile([1, 3], F32, tag="o3", name=_nm("o3"))
                nc.vector.tensor_add(o3[:], h6[:1, 0:3], h6[:1, 3:6])
                nc.vector.tensor_add(o3[:], o3[:], pb_sb[:])
                nc.sync.dma_start(out=out[b : b + 1, :], in_=o3[:])

            for pair in range(BL // 2):
                pending = [batch_program(2 * pair), batch_program(2 * pair + 1)]
                while pending:
                    for g_ in list(pending):
                        try:
                            next(g_)
                        except StopIteration:
                            pending.remove(g_)

    _split_control_waits(nc)
    return nc


# ---------------------------------------------------------------- host side
_CACHE = {}


def _get_nc():
    if "nc" not in _CACHE:
        _CACHE["nc"] = build_nc()
    return _CACHE["nc"]


def kernel(**inputs):
    x_enc = np.asarray(inputs["x_enc"], dtype=np.float32)  # (B, L, C_IN)
    token_w = np.asarray(inputs["token_w"], dtype=np.float32)
    qw = np.asarray(inputs["qw"], dtype=np.float32)
    kw = np.asarray(inputs["kw"], dtype=np.float32)
    vw = np.asarray(inputs["vw"], dtype=np.float32)
    ow = np.asarray(inputs["ow"], dtype=np.float32)
    c1w = np.asarray(inputs["c1w"], dtype=np.float32)
    c2w = np.asarray(inputs["c2w"], dtype=np.float32)
    norm_w = np.asarray(inputs["norm_w"], dtype=np.float32)
    norm_b = np.asarray(inputs["norm_b"], dtype=np.float32)
    proj_w = np.asarray(inputs["proj_w"], dtype=np.float32)
    proj_b = np.asarray(inputs["proj_b"], dtype=np.float32)

    # host-side layout marshalling (no arithmetic)
    tokw = np.ascontiguousarray(token_w.transpose(1, 2, 0).reshape(63, D))
    # xemb[b, c*3+j, l] = x_enc[b, (l+j-1) % L, c]
    xt = x_enc.transpose(0, 2, 1)  # (B, C, L)
    xemb = np.ascontiguousarray(
        np.stack([np.roll(xt, 1 - j, axis=2) for j in range(3)], axis=2).reshape(
            B, 63, L
        )
    )
    shared = {
        "tokw": tokw,
        "wq": np.ascontiguousarray(qw.transpose(0, 2, 1)),
        "wk": np.ascontiguousarray(kw.transpose(0, 2, 1)),
        "wv": np.ascontiguousarray(vw.transpose(0, 2, 1)),
        "wo": np.ascontiguousarray(ow.transpose(0, 2, 1)),
        "wc1": np.ascontiguousarray(c1w.transpose(0, 2, 1)),
        "wc2": np.ascontiguousarray(c2w.transpose(0, 2, 1)),
        "nw": norm_w.reshape(D, 1).copy(),
        "nb": norm_b.reshape(D, 1).copy(),
        "pw": np.ascontiguousarray(
            proj_w.reshape(3, L, D).transpose(2, 0, 1)
        ).astype(ml_dtypes.bfloat16),
        "pb": proj_b.reshape(1, 3).copy(),
        "onescol": np.ones((128, 1), np.float32),
        "onescolf": np.ones((128, 1), np.float32),
        "onesrow": np.ones((1, 128), np.float32),
        "ident": np.eye(128, dtype=np.float32),
    }
    in_maps = []
    for core in range(NCORES):
        m = dict(shared)
        m["xemb"] = np.ascontiguousarray(xemb[core * BL : (core + 1) * BL])
        in_maps.append(m)

    nc = _get_nc()
    res_ = run_bass_kernel_spmd(nc, in_maps, core_ids=list(range(NCORES)))
    out = np.concatenate([res_.results[i]["out"] for i in range(NCORES)], axis=0)
    return out.astype(np.float32)


if __name__ == "__main__":
    import reference

    inputs = reference.setup_inputs()
    got = kernel(**{k: np.asarray(v) for k, v in inputs.items()})
    exp = np.asarray(reference.reference(**inputs))
    rel = np.abs(got - exp).max() / np.abs(exp).max()
    print("Relative error:", rel)



# revision 12
# speedup vs baseline: 1.1278x; 1.1278x over previous
"""Autoformer encoder (B=32, L=1024, D=256, 3 layers) on 8 TRN2 NeuronCores.

Data-parallel over batch (4 batches/core). All matmuls in f32r (full PE
rate, ~1.6e-4 rel err), fp32 residual stream and vector path.

AutoCorrelation without FFT: the lag-correlation
    C[tau] = (1/D) sum_l <q[:, l+tau], k[:, l]>
is computed as F[p, u] = sum_i sum_d k[d, 128i+p] * q2[d, 128i+u]
(PSUM-accumulated matmuls, q2 time-doubled), so that
C[tau] = sum_p F[p, p+tau]. The 128-row shear is done by bouncing F
through DRAM with row stride 1153 and reading back the strided view
[[1154, 128], [1, 1024]]; the partition sum is a ones-vector matmul.
Top-6 lags via vector.max/max_index.

The delay-rolled weighted sum of V uses register-dynamic slices into a
time-doubled V buffer. Each dynamic-AP instruction permanently consumes
~2 registers of the executing engine's 49 (no caching in this build), so
the 72 total gather slots are spread: 23 on ACT (scaled copy), 23 on DVE
(scalar_tensor_tensor FMA), 22 on Pool (FMA), 4 on PE (scaled-identity
matmul with dynamic rhs, PSUM-accumulated).
"""

import contextlib
import numpy as np
import ml_dtypes

import concourse.bass as bass
import concourse.mybir as mybir
from concourse import tile
from concourse.tile import TileContext
from concourse.tile_rust import add_dep_helper
from concourse.vector_clock import ScopedClock
from concourse.bass_utils import run_bass_kernel_spmd

F32 = mybir.dt.float32
F32R = mybir.dt.float32r
BF16 = mybir.dt.bfloat16
U32 = mybir.dt.uint32
AF = mybir.ActivationFunctionType
AX = mybir.AxisListType
ALU = mybir.AluOpType
ET = mybir.EngineType

B, L, C_IN = 32, 1024, 21
D, DFF, NL = 256, 1024, 3
TOPK = 6
NCORES = 8
BL = B // NCORES  # batches per core

HW = 1153  # F bounce row stride (1152 data + 1 pad)
FSH_SZ = 127 * HW + 1152


# ---------------------------------------------------------------- walrus fix
def _patched_drain_and_barrier(self, tick_clock, wait_clock):
    nc = self.nc
    drain_inst = nc.sync.drain()
    wait_clock.add_sem_waits(
        drain_inst.ins, ScopedClock({None: tick_clock.global_clock})
    )
    si = drain_inst.ins.sync_info
    if si is not None and len(si.on_wait) > 1:
        extra = list(si.on_wait[1:])
        del si.on_wait[1:]
        for w in extra:
            n = nc.sync.nop()
            n.ins.sync_info = mybir.SyncInfo(on_update=[], on_wait=[w])
    nc.all_engine_barrier()
    assert self.sems is not None
    popped = nc._tile_sem_poison_stack.pop()
    assert popped is self._sem_poison
    nc.clear_and_free_semaphores(list(self.sems.allocated().values()))
    nc.all_engine_barrier()


tile.TileContext._drain_and_barrier = _patched_drain_and_barrier

_wsctr = [0]


def _split_control_waits(nc):
    """This walrus build allows only ONE sync wait per instruction;
    hoist extras onto NoOps just before, same engine."""
    for fn in nc.m.functions:
        for bb in fn.blocks:
            out = []
            changed = False
            for inst in bb.instructions:
                si = getattr(inst, "sync_info", None)
                if si is not None and len(si.on_wait) > 1:
                    extra = list(si.on_wait[1:])
                    del si.on_wait[1:]
                    for w in extra:
                        _wsctr[0] += 1
                        n = mybir.InstNoOp(
                            name=f"I-waitsplit-{_wsctr[0]}", ins=[], outs=[]
                        )
                        n.engine = inst.engine
                        n.sync_info = mybir.SyncInfo(on_update=[], on_wait=[w])
                        out.append(n)
                        changed = True
                out.append(inst)
            if changed:
                bb.instructions[:] = out


def r(ap):
    return ap


def dep(a, b):
    add_dep_helper(a.ins, b.ins, sync=False, reason="gather order")


# ---------------------------------------------------------------- builder
def build_nc():
    nc = bass.Bass()
    P = lambda name, shape, dt=F32: nc.declare_dram_parameter(
        name, shape, dt, isOutput=False
    )
    xemb = P("xemb", [BL, 63, L], F32R)  # host im2col of token conv input
    tokw = P("tokw", [63, D], F32R)  # lhsT for token conv
    wq = P("wq", [NL, D, D], F32R)  # lhsT (= W.T) per layer
    wk = P("wk", [NL, D, D], F32R)
    wv = P("wv", [NL, D, D], F32R)
    wo = P("wo", [NL, D, D], F32R)
    wc1 = P("wc1", [NL, D, DFF], F32R)  # lhsT
    wc2 = P("wc2", [NL, DFF, D], F32R)  # lhsT
    nwp = P("nw", [D, 1])
    nbp = P("nb", [D, 1])
    pw = P("pw", [D, 3, L], BF16)  # proj_w as [d, class, l], bf16
    pb = P("pb", [1, 3])
    onescol = P("onescol", [128, 1], F32R)
    onescolf = P("onescolf", [128, 1])
    onesrow = P("onesrow", [1, 128])
    ones2d = P("ones2d", [128, 128], F32R)
    negfifth = P("negfifth", [128, 128], F32R)  # -0.2 * I, lhsT for decomp
    ident = P("ident", [128, 128])
    out = nc.declare_dram_parameter("out", [BL, 3], F32, isOutput=True)
    import os
    KDBG = bool(os.environ.get("KDBG"))
    dbg = {}
    if KDBG:
        for nm_, shp, dt_ in [
            ("dbg_x0", [2, 128, L], F32),
            ("dbg_k", [128, L], F32),
            ("dbg_q2", [128, 4096], F32),
            ("dbg_f", [128, 1152], F32),
            ("dbg_h", [128, L], F32),
            ("dbg_c", [1, L], F32),
            ("dbg_ix", [1, 8], U32),
            ("dbg_wb", [128, 8], F32),
            ("dbg_a", [128, 2048], F32),
            ("dbg_s", [2, 128, L + 4], F32),
            ("dbg_x1", [2, 128, L + 4], F32),
            ("dbg_xo", [2, 128, L], F32),
        ]:
            dbg[nm_] = nc.declare_dram_parameter(nm_, shp, dt_, isOutput=True)

    fsh = nc.dram_tensor("fsh", [BL * NL, FSH_SZ], F32R)

    with TileContext(nc) as tc:
        ctx = contextlib.ExitStack()
        with ctx:
            wp = ctx.enter_context(tc.tile_pool(name="weights", bufs=1))
            res = ctx.enter_context(tc.tile_pool(name="res", bufs=6))
            scr = ctx.enter_context(tc.tile_pool(name="scr", bufs=8))
            big = ctx.enter_context(tc.tile_pool(name="big4k", bufs=3))
            gat = ctx.enter_context(tc.tile_pool(name="gat", bufs=3))
            sp = ctx.enter_context(tc.tile_pool(name="small", bufs=4))
            ps = ctx.enter_context(tc.tile_pool(name="psum", bufs=3, space="PSUM"))
            ps2p = ctx.enter_context(
                tc.tile_pool(name="psumB", bufs=2, space="PSUM")
            )

            _names = [0]

            def _nm(pfx):
                _names[0] += 1
                return f"{pfx}{_names[0]}"

            def rtile():
                return res.tile([128, L + 4], F32R, tag="res", name=_nm("rt"))

            def stile(fr=1152, dt=F32, p=128):
                return scr.tile([p, fr], dt, tag="scr", name=_nm("st"))

            # ---- load weights to SBUF once
            tokw_sb = wp.tile([63, D], F32R, tag="tokw")
            nc.sync.dma_start(out=tokw_sb[:], in_=tokw[:])
            ones_sb = wp.tile([128, 1], F32R, tag="ones")
            ones2_sb = wp.tile([128, 1], F32, tag="ones2")
            nc.sync.dma_start(out=ones_sb[:], in_=onescol[:])
            nc.sync.dma_start(out=ones2_sb[:], in_=onescolf[:])
            onesr_sb = wp.tile([1, 128], F32, tag="onesr")
            nc.sync.dma_start(out=onesr_sb[:], in_=onesrow[:])
            ones2d_sb = wp.tile([128, 128], F32R, tag="ones2d")
            nc.sync.dma_start(out=ones2d_sb[:], in_=ones2d[:])
            negf_sb = wp.tile([128, 128], F32R, tag="negf")
            nc.sync.dma_start(out=negf_sb[:], in_=negfifth[:])
            id_sb = wp.tile([128, 128], F32, tag="id")
            nc.sync.dma_start(out=id_sb[:], in_=ident[:])
            nw_sb = wp.tile([128, 2], F32, tag="nw")  # col t = tile t
            nb_sb = wp.tile([128, 2], F32, tag="nb")
            for t in range(2):
                nc.sync.dma_start(
                    out=nw_sb[:, t : t + 1], in_=nwp[t * 128 : (t + 1) * 128, :]
                )
                nc.sync.dma_start(
                    out=nb_sb[:, t : t + 1], in_=nbp[t * 128 : (t + 1) * 128, :]
                )
            pb_sb = wp.tile([1, 3], F32, tag="pb")
            nc.sync.dma_start(out=pb_sb[:], in_=pb[:])

            # layer weights streamed per (b, l), double-buffered
            ws = ctx.enter_context(tc.tile_pool(name="wstream", bufs=2))

            def lload(name, src, l, kt, cols):
                tl = ws.tile(
                    [128, cols], F32R, tag=f"{name}k{kt}", name=_nm(f"{name}{l}")
                )
                nc.sync.dma_start(out=tl[:], in_=src[l, kt * 128 : (kt + 1) * 128, :])
                return tl
            pw_sb = [None, None]
            for t in range(2):
                pw_sb[t] = wp.tile([128, 3 * L], BF16, tag=f"pw{t}", name=f"pw{t}")
                nc.sync.dma_start(
                    out=pw_sb[t][:].rearrange("p (c l) -> p c l", c=3),
                    in_=pw[t * 128 : (t + 1) * 128, :, :],
                )

            # persistent per-engine delay registers + snapped values
            engs = {
                "ACT": nc.engines[ET.Activation],
                "DVE": nc.engines[ET.DVE],
                "POOL": nc.engines[ET.Pool],
                "PE": nc.engines[ET.PE],
            }
            dreg = {k: e.alloc_register(f"dly_{k}") for k, e in engs.items()}
            dval = {
                k: nc.snap(rg, donate=True, min_val=0, max_val=1023)
                for k, rg in dreg.items()
            }

            def proj(dst_fn, w_sb_l, src_aps):
                """dst[mt][chunk] <- sum_kt w[kt].T @ src[kt][:, chunk]."""
                for mt in range(2):
                    for ch in range(2):
                        p5 = ps2p.tile([128, 512], F32, tag="mm512", name=_nm("p5"))
                        for kt in range(2):
                            nc.tensor.matmul(
                                p5[:],
                                r(w_sb_l[kt][:, mt * 128 : (mt + 1) * 128]),
                                r(src_aps[kt][:, ch * 512 : (ch + 1) * 512]),
                                start=(kt == 0),
                                stop=(kt == 1),
                            )
                        dst_fn(mt, ch, p5)

            def batch_program(b):
                # ---- token embedding: x[d, l], 2 tiles, data in [0, L)
                xe_sb = stile(fr=L, p=63, dt=F32R)
                nc.sync.dma_start(out=xe_sb[:], in_=xemb[b, :, :])
                x_sb = [rtile() for _ in range(2)]
                for mt in range(2):
                    for ch in range(2):
                        p5 = ps2p.tile([128, 512], F32, tag="mm512", name=_nm("p5"))
                        nc.tensor.matmul(
                            p5[:],
                            r(tokw_sb[:, mt * 128 : (mt + 1) * 128]),
                            r(xe_sb[:, ch * 512 : (ch + 1) * 512]),
                            start=True,
                            stop=True,
                        )
                        nc.vector.tensor_copy(
                            x_sb[mt][:, ch * 512 : (ch + 1) * 512], p5[:]
                        )

                if KDBG and b == 0:
                    for t in range(2):
                        nc.sync.dma_start(
                            out=dbg["dbg_x0"][t], in_=x_sb[t][:, 0:L].bitcast(F32)
                        )

                for l in range(NL):
                    last_bl = (b == BL - 1) and (l == NL - 1)
                    tap = KDBG and b == 0 and l == 0
                    wq_l = [lload("wq", wq, l, t, D) for t in range(2)]
                    wk_l = [lload("wk", wk, l, t, D) for t in range(2)]
                    wv_l = [lload("wv", wv, l, t, D) for t in range(2)]
                    wo_l = [lload("wo", wo, l, t, D) for t in range(2)]
                    wc1_l = [lload("wc1", wc1, l, t, DFF) for t in range(2)]
                    wc2_l = [lload("wc2", wc2, l, t, D) for t in range(8)]
                    # ---- Q (doubled, stacked kt: col 2048*kt + u), K, V (same)
                    q2_sb = big.tile([128, 4096], F32R, tag="big4k", name=_nm("q2"))
                    v4_sb = big.tile([128, 4096], F32R, tag="big4k", name=_nm("v4"))
                    k_sb = [stile(dt=F32R) for _ in range(2)]

                    def dbl_out(dst):
                        def f(mt, ch, p5):
                            base = 2048 * mt + ch * 512
                            nc.vector.tensor_copy(dst[:, base : base + 512], p5[:])
                            nc.scalar.copy(dst[:, base + 1024 : base + 1536], p5[:])

                        return f

                    def k_out(mt, ch, p5):
                        nc.scalar.copy(
                            k_sb[mt][:, ch * 512 : (ch + 1) * 512], p5[:]
                        )

                    xin = [x_sb[t][:, 0:L] for t in range(2)]
                    proj(dbl_out(q2_sb), wq_l, xin)
                    proj(k_out, wk_l, xin)
                    proj(dbl_out(v4_sb), wv_l, xin)

                    if tap:
                        nc.sync.dma_start(
                            out=dbg["dbg_k"][:], in_=k_sb[0][:, 0:L].bitcast(F32)
                        )
                        nc.sync.dma_start(
                            out=dbg["dbg_q2"][:], in_=q2_sb[:].bitcast(F32)
                        )

                    # ---- F[p, u] = sum_i sum_d k[d,128i+p] q2[d,128i+u]
                    # F in two PSUM tiles so "big" slots stay 2 banks and F
                    # can overlap the FFN's ps2 accumulators. Each 384-wide
                    # chunk is bank-aligned (a matmul output may not cross a
                    # 512-f32 PSUM bank).
                    fps_a = ps.tile([128, 1024], F32, tag="big", name=_nm("fpsa"))
                    fps_b = ps2p.tile([128, 512], F32, tag="mm512", name=_nm("fpsb"))
                    for ch in range(3):  # 3 x 384
                        dstp = (
                            fps_a[:, ch * 512 : ch * 512 + 384]
                            if ch < 2
                            else fps_b[:, 0:384]
                        )
                        for i in range(8):
                            for kt in range(2):
                                base = 2048 * kt + i * 128 + ch * 384
                                nc.tensor.matmul(
                                    dstp,
                                    r(k_sb[kt][:, i * 128 : (i + 1) * 128]),
                                    r(q2_sb[:, base : base + 384]),
                                    start=((i, kt) == (0, 0)),
                                    stop=((i, kt) == (7, 1)),
                                )
                    # bounce through DRAM with the shear stride
                    f_sb = stile(dt=F32R)
                    nc.vector.tensor_copy(
                        f_sb[:, 0:768].rearrange("p (c u) -> p c u", c=2),
                        fps_a[:].rearrange("p (c u) -> p c u", c=2)[:, :, 0:384],
                    )
                    nc.vector.tensor_copy(f_sb[:, 768:1152], fps_b[:, 0:384])
                    frow = fsh[b * NL + l, :]
                    wview = bass.AP(frow.tensor, frow.offset, [[HW, 128], [1, 1152]])
                    fwr = nc.sync.dma_start(out=wview, in_=f_sb[:, 0:1152])
                    hview = bass.AP(
                        frow.tensor, frow.offset, [[HW + 1, 128], [1, 1024]]
                    )
                    h_sb = stile(dt=F32R)
                    hrd = nc.sync.dma_start(out=h_sb[:, 0:1024], in_=hview)
                    add_dep_helper(
                        hrd.ins, fwr.ins, sync=True, reason="hankel read after write"
                    )
                    yield
                    if tap:
                        nc.sync.dma_start(
                            out=dbg["dbg_f"][:], in_=f_sb[:, 0:1152].bitcast(F32)
                        )
                        nc.sync.dma_start(
                            out=dbg["dbg_h"][:], in_=h_sb[:, 0:1024].bitcast(F32)
                        )

                    # ---- C[tau] = (1/256) * sum_p H[p, tau]; top-6; softmax.
                    # All-ones lhsT broadcasts the partition sum to all 128
                    # partitions, so the whole softmax chain runs redundantly
                    # per-partition (same modeled cost: free-size only) and
                    # no PE/PSUM broadcast of the weights is needed.
                    c_sb = stile()
                    for ch in range(2):
                        cp = ps2p.tile([128, 512], F32, tag="mm512", name=_nm("cp"))
                        nc.tensor.matmul(
                            cp[:],
                            r(ones2d_sb[:]),
                            r(h_sb[:, ch * 512 : (ch + 1) * 512]),
                            start=True,
                            stop=True,
                        )
                        nc.scalar.activation(
                            c_sb[:, ch * 512 : (ch + 1) * 512],
                            cp[:],
                            AF.Copy,
                            scale=1.0 / D,
                        )
                    mx = sp.tile([128, 8], F32, tag="mx", name=_nm("mx"))
                    ix = sp.tile([128, 8], U32, tag="ix", name=_nm("ix"))
                    nc.vector.max(out=mx[:], in_=c_sb[:, 0:1024])
                    nc.vector.max_index(
                        out=ix[:], in_max=mx[:], in_values=c_sb[:, 0:1024]
                    )
                    negmax = sp.tile([128, 1], F32, tag="negmax", name=_nm("ng"))
                    nc.vector.tensor_scalar_mul(negmax[:], mx[:, 0:1], -1.0)
                    ex = sp.tile([128, 8], F32, tag="ex", name=_nm("ex"))
                    nc.scalar.activation(
                        ex[:, 0:TOPK], mx[:, 0:TOPK], AF.Exp, bias=negmax[:, 0:1]
                    )
                    esum = sp.tile([128, 1], F32, tag="esum", name=_nm("es"))
                    nc.vector.reduce_sum(esum[:], ex[:, 0:TOPK], axis=AX.X)
                    rinv = sp.tile([128, 1], F32, tag="rinv", name=_nm("ri"))
                    nc.vector.reciprocal(rinv[:], esum[:])
                    wb = sp.tile([128, 8], F32, tag="wb", name=_nm("wb"))
                    nc.vector.tensor_scalar_mul(
                        wb[:, 0:TOPK], ex[:, 0:TOPK], rinv[:, 0:1]
                    )
                    if tap:
                        nc.sync.dma_start(out=dbg["dbg_c"][:], in_=c_sb[:1, 0:L])
                        nc.sync.dma_start(out=dbg["dbg_ix"][:], in_=ix[:1])
                        nc.sync.dma_start(
                            out=dbg["dbg_wb"][:, 0:TOPK], in_=wb[:, 0:TOPK]
                        )

                    # ---- a[:, 1024*t + u] = sum_i w_i V[t][:, (u+d_i) % L]
                    a_sb = gat.tile([128, 2048], F32R, tag="gat", name=_nm("a"))
                    tq_sb = gat.tile([128, 2048], F32R, tag="gat", name=_nm("tq"))
                    pq_sb = gat.tile([128, 2048], F32R, tag="gat", name=_nm("pq"))
                    v4r = v4_sb[:].rearrange("p (b u) -> p b u", b=2)
                    a3 = a_sb[:].rearrange("p (b u) -> p b u", b=2)
                    tq3 = tq_sb[:].rearrange("p (b u) -> p b u", b=2)
                    pq3 = pq_sb[:].rearrange("p (b u) -> p b u", b=2)

                    def ld(ekey, i):
                        return engs[ekey].reg_load(dreg[ekey], ix[:1, i : i + 1])

                    def act_copy(i, dst3):
                        return nc.scalar.activation(
                            dst3,
                            v4r[:, :, bass.ds(dval["ACT"], 1024)],
                            AF.Copy,
                            scale=wb[:, i : i + 1],
                        )

                    def fma(ekey, i):
                        eng = nc.vector if ekey == "DVE" else nc.gpsimd
                        return eng.scalar_tensor_tensor(
                            a3,
                            v4r[:, :, bass.ds(dval[ekey], 1024)],
                            wb[:, i : i + 1],
                            a3,
                            op0=ALU.mult,
                            op1=ALU.add,
                        )

                    if not last_bl:
                        l0 = ld("ACT", 0)
                        o0 = act_copy(0, a3)
                        dep(o0, l0)
                        l1 = ld("ACT", 1)
                        dep(l1, o0)
                        o1 = act_copy(1, tq3)
                        dep(o1, l1)
                        l2 = ld("DVE", 2)
                        o2 = fma("DVE", 2)
                        dep(o2, l2)
                        l3 = ld("DVE", 3)
                        dep(l3, o2)
                        o3_ = fma("DVE", 3)
                        dep(o3_, l3)
                        # Pool: tensor_tensor mult with broadcast weight
                        l4 = ld("POOL", 4)
                        o4 = nc.gpsimd.tensor_mul(
                            pq3,
                            v4r[:, :, bass.ds(dval["POOL"], 1024)],
                            wb[:, 4:5].to_broadcast([128, 2, 1024]),
                        )
                        dep(o4, l4)
                        ad4 = nc.vector.tensor_add(a_sb[:], a_sb[:], pq_sb[:])
                        l5 = ld("POOL", 5)
                        dep(l5, o4)
                        o5 = nc.gpsimd.tensor_mul(
                            pq3,
                            v4r[:, :, bass.ds(dval["POOL"], 1024)],
                            wb[:, 5:6].to_broadcast([128, 2, 1024]),
                        )
                        dep(o5, l5)
                        nc.vector.tensor_add(a_sb[:], a_sb[:], pq_sb[:])
                        nc.vector.tensor_add(a_sb[:], a_sb[:], tq_sb[:])
                    else:
                        # last (b, l): ACT slot 0, DVE slot 1, PE slots 2..5
                        l0 = ld("ACT", 0)
                        o0 = act_copy(0, a3)
                        dep(o0, l0)
                        l1 = ld("DVE", 1)
                        o1 = fma("DVE", 1)
                        dep(o1, l1)
                        pe = engs["PE"]
                        wds = []
                        for i in range(2, 6):
                            wd = stile(fr=128, dt=F32R)
                            nc.vector.tensor_scalar(
                                wd[:, 0:128],
                                id_sb[:],
                                wb[:, i : i + 1],
                                None,
                                op0=ALU.mult,
                            )
                            wds.append(wd)
                        pgs = []
                        prev = None
                        for t in range(2):
                            for c in range(2):
                                pg = ps2p.tile(
                                    [128, 512], F32, tag="mm512", name=_nm("pg")
                                )
                                for ii, i in enumerate(range(2, 6)):
                                    lp = pe.reg_load(dreg["PE"], ix[:1, i : i + 1])
                                    if prev is not None:
                                        dep(lp, prev)
                                    al = pe.reg_alu(
                                        dreg["PE"],
                                        dreg["PE"],
                                        2048 * t + 512 * c,
                                        ALU.add,
                                    )
                                    dep(al, lp)
                                    mm = nc.tensor.matmul(
                                        pg[:],
                                        r(wds[ii][:, 0:128]),
                                        r(v4_sb[:, bass.ds(dval["PE"], 512)]),
                                        start=(ii == 0),
                                        stop=(ii == 3),
                                    )
                                    dep(mm, al)
                                    prev = mm
                                pgs.append((t, c, pg))
                        for t, c, pg in pgs:
                            base = 1024 * t + 512 * c
                            nc.vector.tensor_add(
                                a_sb[:, base : base + 512],
                                a_sb[:, base : base + 512],
                                pg[:],
                            )

                    if tap:
                        nc.sync.dma_start(
                            out=dbg["dbg_a"][:], in_=a_sb[:].bitcast(F32)
                        )

                    yield

                    # ---- O-projection; s = x + a into padded tile (data at 2)
                    s_sb = [rtile() for _ in range(2)]

                    def o_out(mt, ch, p5):
                        nc.vector.tensor_add(
                            s_sb[mt][:, 2 + ch * 512 : 2 + (ch + 1) * 512],
                            x_sb[mt][:, ch * 512 : (ch + 1) * 512],
                            p5[:],
                        )

                    proj(
                        o_out,
                        wo_l,
                        [a_sb[:, 1024 * t : 1024 * (t + 1)] for t in range(2)],
                    )

                    # ---- series_decomp on PE: dst[:, off+u] =
                    # src[2+u] - 0.2*sum_{j=0..4} src[j+u], as 6 PSUM-
                    # accumulated identity matmuls per 512-chunk. dst must
                    # be a different tile than src.
                    def decomp(src_pad, dst, dst_off):
                        # src_pad: [128, 1028] with data in cols [2, 1026)
                        nc.vector.tensor_copy(
                            src_pad[:, 0:2], src_pad[:, 2:3].to_broadcast([128, 2])
                        )
                        nc.vector.tensor_copy(
                            src_pad[:, 1026:1028],
                            src_pad[:, 1025:1026].to_broadcast([128, 2]),
                        )
                        idr = id_sb[:].bitcast(F32R)
                        for c in range(2):
                            pg = ps2p.tile([128, 512], F32, tag="mm512", name=_nm("dc"))
                            nc.tensor.matmul(
                                pg[:],
                                idr,
                                src_pad[:, 2 + c * 512 : 2 + c * 512 + 512],
                                start=True,
                                stop=False,
                            )
                            for j in range(5):
                                nc.tensor.matmul(
                                    pg[:],
                                    negf_sb[:],
                                    src_pad[:, j + c * 512 : j + c * 512 + 512],
                                    start=False,
                                    stop=(j == 4),
                                )
                            nc.scalar.activation(
                                dst[:, dst_off + c * 512 : dst_off + c * 512 + 512],
                                pg[:],
                                AF.Copy,
                            )

                    if tap:
                        for t in range(2):
                            nc.sync.dma_start(
                                out=dbg["dbg_s"][t], in_=s_sb[t][:].bitcast(F32)
                            )

                    # x1 = decomp(s) into x_sb (x is dead once s is formed)
                    for t in range(2):
                        decomp(s_sb[t], x_sb[t], dst_off=2)
                    x1_sb = x_sb
                    if tap:
                        for t in range(2):
                            nc.sync.dma_start(
                                out=dbg["dbg_x1"][t], in_=x1_sb[t][:].bitcast(F32)
                            )

                    # ---- FFN: y = gelu(c1 @ x1); s2 = x1 + c2 @ y (in place)
                    x1v = [x1_sb[t][:, 2:1026] for t in range(2)]
                    ps2 = [
                        ps.tile([128, 1024], F32, tag="big", name=_nm("ps2"))
                        for _ in range(2)
                    ]
                    for ft in range(8):
                        y_sb = stile(dt=F32R)
                        for ch in range(2):
                            p5 = ps2p.tile([128, 512], F32, tag="mm512", name=_nm("p5"))
                            for kt in range(2):
                                nc.tensor.matmul(
                                    p5[:],
                                    r(wc1_l[kt][:, ft * 128 : (ft + 1) * 128]),
                                    r(x1v[kt][:, ch * 512 : (ch + 1) * 512]),
                                    start=(kt == 0),
                                    stop=(kt == 1),
                                )
                            nc.scalar.activation(
                                y_sb[:, ch * 512 : (ch + 1) * 512], p5[:], AF.Gelu
                            )
                        for mt in range(2):
                            for ch in range(2):
                                nc.tensor.matmul(
                                    ps2[mt][:, ch * 512 : (ch + 1) * 512],
                                    r(wc2_l[ft][:, mt * 128 : (mt + 1) * 128]),
                                    r(y_sb[:, ch * 512 : (ch + 1) * 512]),
                                    start=(ft == 0),
                                    stop=(ft == 7),
                                )
                    # s2 = x1 + ffn(x1) into s_sb (s is dead once x1 exists)
                    for mt in range(2):
                        for ch in range(2):
                            nc.vector.tensor_add(
                                s_sb[mt][:, 2 + ch * 512 : 2 + (ch + 1) * 512],
                                x1v[mt][:, ch * 512 : (ch + 1) * 512],
                                ps2[mt][:, ch * 512 : (ch + 1) * 512],
                            )
                    for t in range(2):
                        decomp(s_sb[t], x_sb[t], dst_off=0)
                    yield
                    if tap:
                        for t in range(2):
                            nc.sync.dma_start(
                                out=dbg["dbg_xo"][t], in_=x_sb[t][:, 0:L].bitcast(F32)
                            )

                # ---- my_layernorm + gelu + head
                xv = [x_sb[t][:, 0:L] for t in range(2)]
                xsq = [stile(dt=F32R) for _ in range(2)]
                for t in range(2):
                    nc.scalar.activation(xsq[t][:, 0:L], xv[t], AF.Square)
                mub = stile()
                rstdb = stile()
                for ch in range(2):
                    cs = ps2p.tile([128, 512], F32, tag="mm512", name=_nm("cs"))
                    for kt in range(2):
                        nc.tensor.matmul(
                            cs[:],
                            r(ones2d_sb[:]),
                            r(xv[kt][:, ch * 512 : (ch + 1) * 512]),
                            start=(kt == 0),
                            stop=(kt == 1),
                        )
                    nc.scalar.activation(
                        mub[:, ch * 512 : (ch + 1) * 512], cs[:], AF.Copy, scale=1.0 / D
                    )
                    cq = ps2p.tile([128, 512], F32, tag="mm512", name=_nm("cq"))
                    for kt in range(2):
                        nc.tensor.matmul(
                            cq[:],
                            r(ones2d_sb[:]),
                            r(xsq[kt][:, ch * 512 : (ch + 1) * 512]),
                            start=(kt == 0),
                            stop=(kt == 1),
                        )
                    nc.scalar.activation(
                        rstdb[:, ch * 512 : (ch + 1) * 512],
                        cq[:],
                        AF.Copy,
                        scale=1.0 / D,
                    )
                epsb = sp.tile([128, 1], F32, tag="epsb", name=_nm("ep"))
                nc.vector.memset(epsb[:], 1e-5)
                musq = stile()
                nc.vector.tensor_mul(musq[:, 0:1024], mub[:, 0:1024], mub[:, 0:1024])
                nc.vector.tensor_sub(
                    rstdb[:, 0:1024], rstdb[:, 0:1024], musq[:, 0:1024]
                )
                nc.scalar.activation(
                    rstdb[:, 0:1024], rstdb[:, 0:1024], AF.Sqrt, bias=epsb[:, 0:1]
                )
                nc.vector.reciprocal(rstdb[:, 0:1024], rstdb[:, 0:1024])  # rstd
                g_sb = [stile(dt=BF16) for _ in range(2)]
                for t in range(2):
                    xh = stile()
                    nc.vector.tensor_sub(xh[:, 0:L], xv[t], mub[:, 0:L])
                    nc.vector.tensor_mul(xh[:, 0:L], xh[:, 0:L], rstdb[:, 0:L])
                    nc.scalar.activation(
                        xh[:, 0:L],
                        xh[:, 0:L],
                        AF.Identity,
                        bias=nb_sb[:, t : t + 1],
                        scale=nw_sb[:, t : t + 1],
                    )
                    rowm = sp.tile([128, 1], F32, tag="rowm", name=_nm("rm"))
                    nc.vector.reduce_sum(rowm[:], xh[:, 0:L], axis=AX.X)
                    nc.vector.tensor_scalar_mul(rowm[:], rowm[:], 1.0 / L)
                    nc.vector.tensor_scalar_sub(xh[:, 0:L], xh[:, 0:L], rowm[:, 0:1])
                    nc.scalar.activation(g_sb[t][:, 0:L], xh[:, 0:L], AF.Gelu)

                # head: out[c] = sum_{t,p,l} g[t][p,l] * pw[t][p, c, l] + pb
                hsum = sp.tile([128, 8], F32, tag="hsum", name=_nm("hs"))
                for t in range(2):
                    for c in range(3):
                        hscr = stile()
                        nc.vector.tensor_mul(
                            hscr[:, 0:L],
                            g_sb[t][:, 0:L],
                            pw_sb[t][:, c * L : (c + 1) * L],
                        )
                        nc.vector.reduce_sum(
                            hsum[:, t * 3 + c : t * 3 + c + 1],
                            hscr[:, 0:L],
                            axis=AX.X,
                        )
                psh = ps2p.tile([1, 6], F32, tag="mm512", name=_nm("ph"))
                nc.tensor.matmul(
                    psh[:], ones2_sb[:], hsum[:, 0:6], start=True, stop=True
                )
                h6 = sp.tile([1, 6], F32, tag="h6", name=_nm("h6"))
                nc.vector.tensor_copy(h6[:], psh[:1, 0:6])
                o3 = sp.tile([1, 3], F32, tag="o3", name=_nm("o3"))
                nc.vector.tensor_add(o3[:], h6[:1, 0:3], h6[:1, 3:6])
                nc.vector.tensor_add(o3[:], o3[:], pb_sb[:])
                nc.sync.dma_start(out=out[b : b + 1, :], in_=o3[:])

            # Rolling window of 2 programs, offset by one segment so the
            # DVE-heavy segment (topk+gather) of one program overlaps the
            # PE-heavy segment (QKV/F or FFN) of the other; a finished
            # program is immediately replaced by the next batch.
            progs = [batch_program(b) for b in range(BL)]
            nxt = 0
            active = []

            def _admit():
                nonlocal nxt
                if nxt < BL:
                    active.append(progs[nxt])
                    nxt += 1
                    return True
                return False

            _admit()
            next(active[0])  # phase offset
            _admit()
            while active:
                for g_ in list(active):
                    try:
                        next(g_)
                    except StopIteration:
                        active.remove(g_)
                        _admit()

    _split_control_waits(nc)
    return nc


# ---------------------------------------------------------------- host side
_CACHE = {}


def _get_nc():
    if "nc" not in _CACHE:
        _CACHE["nc"] = build_nc()
    return _CACHE["nc"]


def kernel(**inputs):
    x_enc = np.asarray(inputs["x_enc"], dtype=np.float32)  # (B, L, C_IN)
    token_w = np.asarray(inputs["token_w"], dtype=np.float32)
    qw = np.asarray(inputs["qw"], dtype=np.float32)
    kw = np.asarray(inputs["kw"], dtype=np.float32)
    vw = np.asarray(inputs["vw"], dtype=np.float32)
    ow = np.asarray(inputs["ow"], dtype=np.float32)
    c1w = np.asarray(inputs["c1w"], dtype=np.float32)
    c2w = np.asarray(inputs["c2w"], dtype=np.float32)
    norm_w = np.asarray(inputs["norm_w"], dtype=np.float32)
    norm_b = np.asarray(inputs["norm_b"], dtype=np.float32)
    proj_w = np.asarray(inputs["proj_w"], dtype=np.float32)
    proj_b = np.asarray(inputs["proj_b"], dtype=np.float32)

    # host-side layout marshalling (no arithmetic)
    tokw = np.ascontiguousarray(token_w.transpose(1, 2, 0).reshape(63, D))
    # xemb[b, c*3+j, l] = x_enc[b, (l+j-1) % L, c]
    xt = x_enc.transpose(0, 2, 1)  # (B, C, L)
    xemb = np.ascontiguousarray(
        np.stack([np.roll(xt, 1 - j, axis=2) for j in range(3)], axis=2).reshape(
            B, 63, L
        )
    )
    shared = {
        "tokw": tokw,
        "wq": np.ascontiguousarray(qw.transpose(0, 2, 1)),
        "wk": np.ascontiguousarray(kw.transpose(0, 2, 1)),
        "wv": np.ascontiguousarray(vw.transpose(0, 2, 1)),
        "wo": np.ascontiguousarray(ow.transpose(0, 2, 1)),
        "wc1": np.ascontiguousarray(c1w.transpose(0, 2, 1)),
        "wc2": np.ascontiguousarray(c2w.transpose(0, 2, 1)),
        "nw": norm_w.reshape(D, 1).copy(),
        "nb": norm_b.reshape(D, 1).copy(),
        "pw": np.ascontiguousarray(
            proj_w.reshape(3, L, D).transpose(2, 0, 1)
        ).astype(ml_dtypes.bfloat16),
        "pb": proj_b.reshape(1, 3).copy(),
        "onescol": np.ones((128, 1), np.float32),
        "onescolf": np.ones((128, 1), np.float32),
        "onesrow": np.ones((1, 128), np.float32),
        "ones2d": np.ones((128, 128), np.float32),
        "ident": np.eye(128, dtype=np.float32),
    }
    in_maps = []
    for core in range(NCORES):
        m = dict(shared)
        m["xemb"] = np.ascontiguousarray(xemb[core * BL : (core + 1) * BL])
        in_maps.append(m)

    nc = _get_nc()
    res_ = run_bass_kernel_spmd(nc, in_maps, core_ids=list(range(NCORES)))
    out = np.concatenate([res_.results[i]["out"] for i in range(NCORES)], axis=0)
    return out.astype(np.float32)


if __name__ == "__main__":
    import reference

    inputs = reference.setup_inputs()
    got = kernel(**{k: np.asarray(v) for k, v in inputs.items()})
    exp = np.asarray(reference.reference(**inputs))
    rel = np.abs(got - exp).max() / np.abs(exp).max()
    print("Relative error:", rel)



# revision 14
# speedup vs baseline: 1.2467x; 1.1054x over previous
"""Autoformer encoder (B=32, L=1024, D=256, 3 layers) on 8 TRN2 NeuronCores.

Data-parallel over batch (4 batches/core). All matmuls in f32r (full PE
rate, ~1.6e-4 rel err), fp32 residual stream and vector path.

AutoCorrelation without FFT: the lag-correlation
    C[tau] = (1/D) sum_l <q[:, l+tau], k[:, l]>
is computed as F[p, u] = sum_i sum_d k[d, 128i+p] * q2[d, 128i+u]
(PSUM-accumulated matmuls, q2 time-doubled), so that
C[tau] = sum_p F[p, p+tau]. The 128-row shear is done by bouncing F
through DRAM with row stride 1153 and reading back the strided view
[[1154, 128], [1, 1024]]; the partition sum is a ones-vector matmul.
Top-6 lags via vector.max/max_index.

The delay-rolled weighted sum of V uses register-dynamic slices into a
time-doubled V buffer. Each dynamic-AP instruction permanently consumes
~2 registers of the executing engine's 49 (no caching in this build), so
the 72 total gather slots are spread: 23 on ACT (scaled copy), 23 on DVE
(scalar_tensor_tensor FMA), 22 on Pool (FMA), 4 on PE (scaled-identity
matmul with dynamic rhs, PSUM-accumulated).
"""

import contextlib
import numpy as np
import ml_dtypes

import concourse.bass as bass
import concourse.mybir as mybir
from concourse import tile
from concourse.tile import TileContext
from concourse.tile_rust import add_dep_helper
from concourse.vector_clock import ScopedClock
from concourse.bass_utils import run_bass_kernel_spmd

F32 = mybir.dt.float32
F32R = mybir.dt.float32r
BF16 = mybir.dt.bfloat16
U32 = mybir.dt.uint32
AF = mybir.ActivationFunctionType
AX = mybir.AxisListType
ALU = mybir.AluOpType
ET = mybir.EngineType

B, L, C_IN = 32, 1024, 21
D, DFF, NL = 256, 1024, 3
TOPK = 6
NCORES = 8
BL = B // NCORES  # batches per core

HW = 1153  # F bounce row stride (1152 data + 1 pad)
FSH_SZ = 127 * HW + 1152


# ---------------------------------------------------------------- walrus fix
def _patched_drain_and_barrier(self, tick_clock, wait_clock):
    nc = self.nc
    drain_inst = nc.sync.drain()
    wait_clock.add_sem_waits(
        drain_inst.ins, ScopedClock({None: tick_clock.global_clock})
    )
    si = drain_inst.ins.sync_info
    if si is not None and len(si.on_wait) > 1:
        extra = list(si.on_wait[1:])
        del si.on_wait[1:]
        for w in extra:
            n = nc.sync.nop()
            n.ins.sync_info = mybir.SyncInfo(on_update=[], on_wait=[w])
    nc.all_engine_barrier()
    assert self.sems is not None
    popped = nc._tile_sem_poison_stack.pop()
    assert popped is self._sem_poison
    nc.clear_and_free_semaphores(list(self.sems.allocated().values()))
    nc.all_engine_barrier()


tile.TileContext._drain_and_barrier = _patched_drain_and_barrier

_wsctr = [0]


def _split_control_waits(nc):
    """This walrus build allows only ONE sync wait per instruction;
    hoist extras onto NoOps just before, same engine."""
    for fn in nc.m.functions:
        for bb in fn.blocks:
            out = []
            changed = False
            for inst in bb.instructions:
                si = getattr(inst, "sync_info", None)
                if si is not None and len(si.on_wait) > 1:
                    extra = list(si.on_wait[1:])
                    del si.on_wait[1:]
                    for w in extra:
                        _wsctr[0] += 1
                        n = mybir.InstNoOp(
                            name=f"I-waitsplit-{_wsctr[0]}", ins=[], outs=[]
                        )
                        n.engine = inst.engine
                        n.sync_info = mybir.SyncInfo(on_update=[], on_wait=[w])
                        out.append(n)
                        changed = True
                out.append(inst)
            if changed:
                bb.instructions[:] = out


def r(ap):
    return ap


def dep(a, b):
    add_dep_helper(a.ins, b.ins, sync=False, reason="gather order")


# ---------------------------------------------------------------- builder
def build_nc():
    nc = bass.Bass()
    P = lambda name, shape, dt=F32: nc.declare_dram_parameter(
        name, shape, dt, isOutput=False
    )
    xemb = P("xemb", [BL, 63, L], F32R)  # host im2col of token conv input
    tokw = P("tokw", [63, D], F32R)  # lhsT for token conv
    wq = P("wq", [NL, D, D], F32R)  # lhsT (= W.T) per layer
    wk = P("wk", [NL, D, D], F32R)
    wv = P("wv", [NL, D, D], F32R)
    wo = P("wo", [NL, D, D], F32R)
    wc1 = P("wc1", [NL, D, DFF], F32R)  # lhsT
    wc2 = P("wc2", [NL, DFF, D], F32R)  # lhsT
    nwp = P("nw", [D, 1])
    nbp = P("nb", [D, 1])
    pw = P("pw", [D, 3, L], BF16)  # proj_w as [d, class, l], bf16
    pb = P("pb", [1, 3])
    onescol = P("onescol", [128, 1], F32R)
    onescolf = P("onescolf", [128, 1])
    onesrow = P("onesrow", [1, 128])
    ones2d = P("ones2d", [128, 128], F32R)
    negfifth = P("negfifth", [128, 128], F32R)  # -0.2 * I, lhsT for decomp
    identr = P("identr", [128, 128], F32R)  # I, lhsT for decomp
    ident = P("ident", [128, 128])
    out = nc.declare_dram_parameter("out", [BL, 3], F32, isOutput=True)
    import os
    KDBG = bool(os.environ.get("KDBG"))
    dbg = {}
    if KDBG:
        for nm_, shp, dt_ in [
            ("dbg_x0", [2, 128, L], F32),
            ("dbg_k", [128, L], F32),
            ("dbg_q2", [128, 4096], F32),
            ("dbg_f", [128, 1152], F32),
            ("dbg_h", [128, L], F32),
            ("dbg_c", [1, L], F32),
            ("dbg_ix", [1, 8], U32),
            ("dbg_wb", [128, 8], F32),
            ("dbg_a", [128, 2048], F32),
            ("dbg_s", [2, 128, L + 4], F32),
            ("dbg_x1", [2, 128, L + 4], F32),
            ("dbg_xo", [2, 128, L], F32),
        ]:
            dbg[nm_] = nc.declare_dram_parameter(nm_, shp, dt_, isOutput=True)

    fsh = nc.dram_tensor("fsh", [BL * NL, FSH_SZ], F32R)

    with TileContext(nc) as tc:
        ctx = contextlib.ExitStack()
        with ctx:
            wp = ctx.enter_context(tc.tile_pool(name="weights", bufs=1))
            res = ctx.enter_context(tc.tile_pool(name="res", bufs=6))
            scr = ctx.enter_context(tc.tile_pool(name="scr", bufs=8))
            big = ctx.enter_context(tc.tile_pool(name="big4k", bufs=3))
            gat = ctx.enter_context(tc.tile_pool(name="gat", bufs=3))
            sp = ctx.enter_context(tc.tile_pool(name="small", bufs=4))
            ps = ctx.enter_context(tc.tile_pool(name="psum", bufs=3, space="PSUM"))
            ps2p = ctx.enter_context(
                tc.tile_pool(name="psumB", bufs=2, space="PSUM")
            )

            _names = [0]

            def _nm(pfx):
                _names[0] += 1
                return f"{pfx}{_names[0]}"

            def rtile():
                return res.tile([128, L + 4], F32R, tag="res", name=_nm("rt"))

            def stile(fr=1152, dt=F32, p=128):
                return scr.tile([p, fr], dt, tag="scr", name=_nm("st"))

            # ---- load weights to SBUF once
            tokw_sb = wp.tile([63, D], F32R, tag="tokw")
            nc.sync.dma_start(out=tokw_sb[:], in_=tokw[:])
            ones_sb = wp.tile([128, 1], F32R, tag="ones")
            ones2_sb = wp.tile([128, 1], F32, tag="ones2")
            nc.sync.dma_start(out=ones_sb[:], in_=onescol[:])
            nc.sync.dma_start(out=ones2_sb[:], in_=onescolf[:])
            onesr_sb = wp.tile([1, 128], F32, tag="onesr")
            nc.sync.dma_start(out=onesr_sb[:], in_=onesrow[:])
            ones2d_sb = wp.tile([128, 128], F32R, tag="ones2d")
            nc.sync.dma_start(out=ones2d_sb[:], in_=ones2d[:])
            negf_sb = wp.tile([128, 128], F32R, tag="negf")
            nc.sync.dma_start(out=negf_sb[:], in_=negfifth[:])
            idr_sb = wp.tile([128, 128], F32R, tag="idr")
            nc.sync.dma_start(out=idr_sb[:], in_=identr[:])
            id_sb = wp.tile([128, 128], F32, tag="id")
            nc.sync.dma_start(out=id_sb[:], in_=ident[:])
            nw_sb = wp.tile([128, 2], F32, tag="nw")  # col t = tile t
            nb_sb = wp.tile([128, 2], F32, tag="nb")
            for t in range(2):
                nc.sync.dma_start(
                    out=nw_sb[:, t : t + 1], in_=nwp[t * 128 : (t + 1) * 128, :]
                )
                nc.sync.dma_start(
                    out=nb_sb[:, t : t + 1], in_=nbp[t * 128 : (t + 1) * 128, :]
                )
            pb_sb = wp.tile([1, 3], F32, tag="pb")
            nc.sync.dma_start(out=pb_sb[:], in_=pb[:])

            # layer weights streamed per (b, l), double-buffered
            ws = ctx.enter_context(tc.tile_pool(name="wstream", bufs=2))

            def lload(name, src, l, kt, cols):
                tl = ws.tile(
                    [128, cols], F32R, tag=f"{name}k{kt}", name=_nm(f"{name}{l}")
                )
                nc.sync.dma_start(out=tl[:], in_=src[l, kt * 128 : (kt + 1) * 128, :])
                return tl
            pw_sb = [None, None]
            for t in range(2):
                pw_sb[t] = wp.tile([128, 3 * L], BF16, tag=f"pw{t}", name=f"pw{t}")
                nc.sync.dma_start(
                    out=pw_sb[t][:].rearrange("p (c l) -> p c l", c=3),
                    in_=pw[t * 128 : (t + 1) * 128, :, :],
                )

            # persistent per-engine delay registers + snapped values
            engs = {
                "ACT": nc.engines[ET.Activation],
                "DVE": nc.engines[ET.DVE],
                "POOL": nc.engines[ET.Pool],
                "PE": nc.engines[ET.PE],
            }
            dreg = {k: e.alloc_register(f"dly_{k}") for k, e in engs.items()}
            dval = {
                k: nc.snap(rg, donate=True, min_val=0, max_val=1023)
                for k, rg in dreg.items()
            }

            def proj(dst_fn, w_sb_l, src_aps):
                """dst[mt][chunk] <- sum_kt w[kt].T @ src[kt][:, chunk]."""
                for mt in range(2):
                    for ch in range(2):
                        p5 = ps2p.tile([128, 512], F32, tag="mm512", name=_nm("p5"))
                        for kt in range(2):
                            nc.tensor.matmul(
                                p5[:],
                                r(w_sb_l[kt][:, mt * 128 : (mt + 1) * 128]),
                                r(src_aps[kt][:, ch * 512 : (ch + 1) * 512]),
                                start=(kt == 0),
                                stop=(kt == 1),
                            )
                        dst_fn(mt, ch, p5)

            def batch_program(b):
                # ---- token embedding: x[d, l], 2 tiles, data in [0, L)
                xe_sb = stile(fr=L, p=63, dt=F32R)
                nc.sync.dma_start(out=xe_sb[:], in_=xemb[b, :, :])
                x_sb = [rtile() for _ in range(2)]
                for mt in range(2):
                    for ch in range(2):
                        p5 = ps2p.tile([128, 512], F32, tag="mm512", name=_nm("p5"))
                        nc.tensor.matmul(
                            p5[:],
                            r(tokw_sb[:, mt * 128 : (mt + 1) * 128]),
                            r(xe_sb[:, ch * 512 : (ch + 1) * 512]),
                            start=True,
                            stop=True,
                        )
                        nc.vector.tensor_copy(
                            x_sb[mt][:, ch * 512 : (ch + 1) * 512], p5[:]
                        )

                if KDBG and b == 0:
                    for t in range(2):
                        nc.sync.dma_start(
                            out=dbg["dbg_x0"][t], in_=x_sb[t][:, 0:L].bitcast(F32)
                        )

                for l in range(NL):
                    last_bl = (b == BL - 1) and (l == NL - 1)
                    tap = KDBG and b == 0 and l == 0
                    wq_l = [lload("wq", wq, l, t, D) for t in range(2)]
                    wk_l = [lload("wk", wk, l, t, D) for t in range(2)]
                    wv_l = [lload("wv", wv, l, t, D) for t in range(2)]
                    wo_l = [lload("wo", wo, l, t, D) for t in range(2)]
                    wc1_l = [lload("wc1", wc1, l, t, DFF) for t in range(2)]
                    wc2_l = [lload("wc2", wc2, l, t, D) for t in range(8)]
                    # ---- Q (doubled, stacked kt: col 2048*kt + u), K, V (same)
                    q2_sb = big.tile([128, 4096], F32R, tag="big4k", name=_nm("q2"))
                    v4_sb = big.tile([128, 4096], F32R, tag="big4k", name=_nm("v4"))
                    k_sb = [stile(dt=F32R) for _ in range(2)]

                    def dbl_out(dst):
                        def f(mt, ch, p5):
                            base = 2048 * mt + ch * 512
                            nc.vector.tensor_copy(dst[:, base : base + 512], p5[:])
                            nc.scalar.copy(dst[:, base + 1024 : base + 1536], p5[:])

                        return f

                    def k_out(mt, ch, p5):
                        nc.scalar.copy(
                            k_sb[mt][:, ch * 512 : (ch + 1) * 512], p5[:]
                        )

                    xin = [x_sb[t][:, 0:L] for t in range(2)]
                    proj(dbl_out(q2_sb), wq_l, xin)
                    proj(k_out, wk_l, xin)
                    proj(dbl_out(v4_sb), wv_l, xin)

                    if tap:
                        nc.sync.dma_start(
                            out=dbg["dbg_k"][:], in_=k_sb[0][:, 0:L].bitcast(F32)
                        )
                        nc.sync.dma_start(
                            out=dbg["dbg_q2"][:], in_=q2_sb[:].bitcast(F32)
                        )

                    # ---- F[p, u] = sum_i sum_d k[d,128i+p] q2[d,128i+u]
                    # F in two PSUM tiles so "big" slots stay 2 banks and F
                    # can overlap the FFN's ps2 accumulators. Each 384-wide
                    # chunk is bank-aligned (a matmul output may not cross a
                    # 512-f32 PSUM bank).
                    fps_a = ps.tile([128, 1024], F32, tag="big", name=_nm("fpsa"))
                    fps_b = ps2p.tile([128, 512], F32, tag="mm512", name=_nm("fpsb"))
                    for ch in range(3):  # 3 x 384
                        dstp = (
                            fps_a[:, ch * 512 : ch * 512 + 384]
                            if ch < 2
                            else fps_b[:, 0:384]
                        )
                        for i in range(8):
                            for kt in range(2):
                                base = 2048 * kt + i * 128 + ch * 384
                                nc.tensor.matmul(
                                    dstp,
                                    r(k_sb[kt][:, i * 128 : (i + 1) * 128]),
                                    r(q2_sb[:, base : base + 384]),
                                    start=((i, kt) == (0, 0)),
                                    stop=((i, kt) == (7, 1)),
                                )
                    # bounce through DRAM with the shear stride
                    f_sb = stile(dt=F32R)
                    nc.vector.tensor_copy(
                        f_sb[:, 0:768].rearrange("p (c u) -> p c u", c=2),
                        fps_a[:].rearrange("p (c u) -> p c u", c=2)[:, :, 0:384],
                    )
                    nc.vector.tensor_copy(f_sb[:, 768:1152], fps_b[:, 0:384])
                    frow = fsh[b * NL + l, :]
                    wview = bass.AP(frow.tensor, frow.offset, [[HW, 128], [1, 1152]])
                    fwr = nc.sync.dma_start(out=wview, in_=f_sb[:, 0:1152])
                    hview = bass.AP(
                        frow.tensor, frow.offset, [[HW + 1, 128], [1, 1024]]
                    )
                    h_sb = stile(dt=F32R)
                    hrd = nc.sync.dma_start(out=h_sb[:, 0:1024], in_=hview)
                    add_dep_helper(
                        hrd.ins, fwr.ins, sync=True, reason="hankel read after write"
                    )
                    yield
                    if tap:
                        nc.sync.dma_start(
                            out=dbg["dbg_f"][:], in_=f_sb[:, 0:1152].bitcast(F32)
                        )
                        nc.sync.dma_start(
                            out=dbg["dbg_h"][:], in_=h_sb[:, 0:1024].bitcast(F32)
                        )

                    # ---- C[tau] = (1/256) * sum_p H[p, tau]; top-6; softmax.
                    # All-ones lhsT broadcasts the partition sum to all 128
                    # partitions, so the whole softmax chain runs redundantly
                    # per-partition (same modeled cost: free-size only) and
                    # no PE/PSUM broadcast of the weights is needed.
                    c_sb = stile()
                    for ch in range(2):
                        cp = ps2p.tile([128, 512], F32, tag="mm512", name=_nm("cp"))
                        nc.tensor.matmul(
                            cp[:],
                            r(ones2d_sb[:]),
                            r(h_sb[:, ch * 512 : (ch + 1) * 512]),
                            start=True,
                            stop=True,
                        )
                        nc.scalar.activation(
                            c_sb[:, ch * 512 : (ch + 1) * 512],
                            cp[:],
                            AF.Copy,
                            scale=1.0 / D,
                        )
                    mx = sp.tile([128, 8], F32, tag="mx", name=_nm("mx"))
                    ix = sp.tile([128, 8], U32, tag="ix", name=_nm("ix"))
                    nc.vector.max(out=mx[:], in_=c_sb[:, 0:1024])
                    nc.vector.max_index(
                        out=ix[:], in_max=mx[:], in_values=c_sb[:, 0:1024]
                    )
                    negmax = sp.tile([128, 1], F32, tag="negmax", name=_nm("ng"))
                    nc.vector.tensor_scalar_mul(negmax[:], mx[:, 0:1], -1.0)
                    ex = sp.tile([128, 8], F32, tag="ex", name=_nm("ex"))
                    nc.scalar.activation(
                        ex[:, 0:TOPK], mx[:, 0:TOPK], AF.Exp, bias=negmax[:, 0:1]
                    )
                    esum = sp.tile([128, 1], F32, tag="esum", name=_nm("es"))
                    nc.vector.reduce_sum(esum[:], ex[:, 0:TOPK], axis=AX.X)
                    rinv = sp.tile([128, 1], F32, tag="rinv", name=_nm("ri"))
                    nc.vector.reciprocal(rinv[:], esum[:])
                    wb = sp.tile([128, 8], F32, tag="wb", name=_nm("wb"))
                    nc.vector.tensor_scalar_mul(
                        wb[:, 0:TOPK], ex[:, 0:TOPK], rinv[:, 0:1]
                    )
                    if tap:
                        nc.sync.dma_start(out=dbg["dbg_c"][:], in_=c_sb[:1, 0:L])
                        nc.sync.dma_start(out=dbg["dbg_ix"][:], in_=ix[:1])
                        nc.sync.dma_start(
                            out=dbg["dbg_wb"][:, 0:TOPK], in_=wb[:, 0:TOPK]
                        )

                    # ---- a[:, 1024*t + u] = sum_i w_i V[t][:, (u+d_i) % L]
                    a_sb = gat.tile([128, 2048], F32R, tag="gat", name=_nm("a"))
                    tq_sb = gat.tile([128, 2048], F32R, tag="gat", name=_nm("tq"))
                    pq_sb = gat.tile([128, 2048], F32R, tag="gat", name=_nm("pq"))
                    v4r = v4_sb[:].rearrange("p (b u) -> p b u", b=2)
                    a3 = a_sb[:].rearrange("p (b u) -> p b u", b=2)
                    tq3 = tq_sb[:].rearrange("p (b u) -> p b u", b=2)
                    pq3 = pq_sb[:].rearrange("p (b u) -> p b u", b=2)

                    def ld(ekey, i):
                        return engs[ekey].reg_load(dreg[ekey], ix[:1, i : i + 1])

                    def act_copy(i, dst3):
                        return nc.scalar.activation(
                            dst3,
                            v4r[:, :, bass.ds(dval["ACT"], 1024)],
                            AF.Copy,
                            scale=wb[:, i : i + 1],
                        )

                    def fma(ekey, i):
                        eng = nc.vector if ekey == "DVE" else nc.gpsimd
                        return eng.scalar_tensor_tensor(
                            a3,
                            v4r[:, :, bass.ds(dval[ekey], 1024)],
                            wb[:, i : i + 1],
                            a3,
                            op0=ALU.mult,
                            op1=ALU.add,
                        )

                    if not last_bl:
                        l0 = ld("ACT", 0)
                        o0 = act_copy(0, a3)
                        dep(o0, l0)
                        l1 = ld("ACT", 1)
                        dep(l1, o0)
                        o1 = act_copy(1, tq3)
                        dep(o1, l1)
                        l2 = ld("DVE", 2)
                        o2 = fma("DVE", 2)
                        dep(o2, l2)
                        l3 = ld("DVE", 3)
                        dep(l3, o2)
                        o3_ = fma("DVE", 3)
                        dep(o3_, l3)
                        # Pool: tensor_tensor mult with broadcast weight
                        l4 = ld("POOL", 4)
                        o4 = nc.gpsimd.tensor_mul(
                            pq3,
                            v4r[:, :, bass.ds(dval["POOL"], 1024)],
                            wb[:, 4:5].to_broadcast([128, 2, 1024]),
                        )
                        dep(o4, l4)
                        ad4 = nc.vector.tensor_add(a_sb[:], a_sb[:], pq_sb[:])
                        l5 = ld("POOL", 5)
                        dep(l5, o4)
                        o5 = nc.gpsimd.tensor_mul(
                            pq3,
                            v4r[:, :, bass.ds(dval["POOL"], 1024)],
                            wb[:, 5:6].to_broadcast([128, 2, 1024]),
                        )
                        dep(o5, l5)
                        nc.vector.tensor_add(a_sb[:], a_sb[:], pq_sb[:])
                        nc.vector.tensor_add(a_sb[:], a_sb[:], tq_sb[:])
                    else:
                        # last (b, l): ACT slot 0, DVE slot 1, PE slots 2..5
                        l0 = ld("ACT", 0)
                        o0 = act_copy(0, a3)
                        dep(o0, l0)
                        l1 = ld("DVE", 1)
                        o1 = fma("DVE", 1)
                        dep(o1, l1)
                        pe = engs["PE"]
                        wds = []
                        for i in range(2, 6):
                            wd = stile(fr=128, dt=F32R)
                            nc.vector.tensor_scalar(
                                wd[:, 0:128],
                                id_sb[:],
                                wb[:, i : i + 1],
                                None,
                                op0=ALU.mult,
                            )
                            wds.append(wd)
                        pgs = []
                        prev = None
                        for t in range(2):
                            for c in range(2):
                                pg = ps2p.tile(
                                    [128, 512], F32, tag="mm512", name=_nm("pg")
                                )
                                for ii, i in enumerate(range(2, 6)):
                                    lp = pe.reg_load(dreg["PE"], ix[:1, i : i + 1])
                                    if prev is not None:
                                        dep(lp, prev)
                                    al = pe.reg_alu(
                                        dreg["PE"],
                                        dreg["PE"],
                                        2048 * t + 512 * c,
                                        ALU.add,
                                    )
                                    dep(al, lp)
                                    mm = nc.tensor.matmul(
                                        pg[:],
                                        r(wds[ii][:, 0:128]),
                                        r(v4_sb[:, bass.ds(dval["PE"], 512)]),
                                        start=(ii == 0),
                                        stop=(ii == 3),
                                    )
                                    dep(mm, al)
                                    prev = mm
                                pgs.append((t, c, pg))
                        for t, c, pg in pgs:
                            base = 1024 * t + 512 * c
                            nc.vector.tensor_add(
                                a_sb[:, base : base + 512],
                                a_sb[:, base : base + 512],
                                pg[:],
                            )

                    if tap:
                        nc.sync.dma_start(
                            out=dbg["dbg_a"][:], in_=a_sb[:].bitcast(F32)
                        )

                    yield

                    # ---- O-projection; s = x + a into padded tile (data at 2)
                    s_sb = [rtile() for _ in range(2)]

                    def o_out(mt, ch, p5):
                        nc.vector.tensor_add(
                            s_sb[mt][:, 2 + ch * 512 : 2 + (ch + 1) * 512],
                            x_sb[mt][:, ch * 512 : (ch + 1) * 512],
                            p5[:],
                        )

                    proj(
                        o_out,
                        wo_l,
                        [a_sb[:, 1024 * t : 1024 * (t + 1)] for t in range(2)],
                    )

                    # ---- series_decomp on PE: dst[:, off+u] =
                    # src[2+u] - 0.2*sum_{j=0..4} src[j+u], as 6 PSUM-
                    # accumulated identity matmuls per 512-chunk. dst must
                    # be a different tile than src.
                    def decomp(src_pad, dst, dst_off):
                        # src_pad: [128, 1028] with data in cols [2, 1026)
                        nc.vector.tensor_copy(
                            src_pad[:, 0:2], src_pad[:, 2:3].to_broadcast([128, 2])
                        )
                        nc.vector.tensor_copy(
                            src_pad[:, 1026:1028],
                            src_pad[:, 1025:1026].to_broadcast([128, 2]),
                        )
                        idr = idr_sb[:]
                        for c in range(2):
                            pg = ps2p.tile([128, 512], F32, tag="mm512", name=_nm("dc"))
                            nc.tensor.matmul(
                                pg[:],
                                idr,
                                src_pad[:, 2 + c * 512 : 2 + c * 512 + 512],
                                start=True,
                                stop=False,
                            )
                            for j in range(5):
                                nc.tensor.matmul(
                                    pg[:],
                                    negf_sb[:],
                                    src_pad[:, j + c * 512 : j + c * 512 + 512],
                                    start=False,
                                    stop=(j == 4),
                                )
                            nc.scalar.activation(
                                dst[:, dst_off + c * 512 : dst_off + c * 512 + 512],
                                pg[:],
                                AF.Copy,
                            )

                    if tap:
                        for t in range(2):
                            nc.sync.dma_start(
                                out=dbg["dbg_s"][t], in_=s_sb[t][:].bitcast(F32)
                            )

                    # x1 = decomp(s) into x_sb (x is dead once s is formed)
                    for t in range(2):
                        decomp(s_sb[t], x_sb[t], dst_off=2)
                    x1_sb = x_sb
                    if tap:
                        for t in range(2):
                            nc.sync.dma_start(
                                out=dbg["dbg_x1"][t], in_=x1_sb[t][:].bitcast(F32)
                            )

                    # ---- FFN: y = gelu(c1 @ x1); s2 = x1 + c2 @ y (in place)
                    x1v = [x1_sb[t][:, 2:1026] for t in range(2)]
                    ps2 = [
                        ps.tile([128, 1024], F32, tag="big", name=_nm("ps2"))
                        for _ in range(2)
                    ]
                    for ft in range(8):
                        y_sb = stile(dt=F32R)
                        for ch in range(2):
                            p5 = ps2p.tile([128, 512], F32, tag="mm512", name=_nm("p5"))
                            for kt in range(2):
                                nc.tensor.matmul(
                                    p5[:],
                                    r(wc1_l[kt][:, ft * 128 : (ft + 1) * 128]),
                                    r(x1v[kt][:, ch * 512 : (ch + 1) * 512]),
                                    start=(kt == 0),
                                    stop=(kt == 1),
                                )
                            nc.scalar.activation(
                                y_sb[:, ch * 512 : (ch + 1) * 512], p5[:], AF.Gelu
                            )
                        for mt in range(2):
                            for ch in range(2):
                                nc.tensor.matmul(
                                    ps2[mt][:, ch * 512 : (ch + 1) * 512],
                                    r(wc2_l[ft][:, mt * 128 : (mt + 1) * 128]),
                                    r(y_sb[:, ch * 512 : (ch + 1) * 512]),
                                    start=(ft == 0),
                                    stop=(ft == 7),
                                )
                    # s2 = x1 + ffn(x1) into s_sb (s is dead once x1 exists)
                    for mt in range(2):
                        for ch in range(2):
                            nc.vector.tensor_add(
                                s_sb[mt][:, 2 + ch * 512 : 2 + (ch + 1) * 512],
                                x1v[mt][:, ch * 512 : (ch + 1) * 512],
                                ps2[mt][:, ch * 512 : (ch + 1) * 512],
                            )
                    for t in range(2):
                        decomp(s_sb[t], x_sb[t], dst_off=0)
                    yield
                    if tap:
                        for t in range(2):
                            nc.sync.dma_start(
                                out=dbg["dbg_xo"][t], in_=x_sb[t][:, 0:L].bitcast(F32)
                            )

                # ---- my_layernorm + gelu + head
                xv = [x_sb[t][:, 0:L] for t in range(2)]
                xsq = [stile(dt=F32R) for _ in range(2)]
                for t in range(2):
                    nc.scalar.activation(xsq[t][:, 0:L], xv[t], AF.Square)
                mub = stile()
                rstdb = stile()
                for ch in range(2):
                    cs = ps2p.tile([128, 512], F32, tag="mm512", name=_nm("cs"))
                    for kt in range(2):
                        nc.tensor.matmul(
                            cs[:],
                            r(ones2d_sb[:]),
                            r(xv[kt][:, ch * 512 : (ch + 1) * 512]),
                            start=(kt == 0),
                            stop=(kt == 1),
                        )
                    nc.scalar.activation(
                        mub[:, ch * 512 : (ch + 1) * 512], cs[:], AF.Copy, scale=1.0 / D
                    )
                    cq = ps2p.tile([128, 512], F32, tag="mm512", name=_nm("cq"))
                    for kt in range(2):
                        nc.tensor.matmul(
                            cq[:],
                            r(ones2d_sb[:]),
                            r(xsq[kt][:, ch * 512 : (ch + 1) * 512]),
                            start=(kt == 0),
                            stop=(kt == 1),
                        )
                    nc.scalar.activation(
                        rstdb[:, ch * 512 : (ch + 1) * 512],
                        cq[:],
                        AF.Copy,
                        scale=1.0 / D,
                    )
                epsb = sp.tile([128, 1], F32, tag="epsb", name=_nm("ep"))
                nc.vector.memset(epsb[:], 1e-5)
                musq = stile()
                nc.vector.tensor_mul(musq[:, 0:1024], mub[:, 0:1024], mub[:, 0:1024])
                nc.vector.tensor_sub(
                    rstdb[:, 0:1024], rstdb[:, 0:1024], musq[:, 0:1024]
                )
                nc.scalar.activation(
                    rstdb[:, 0:1024], rstdb[:, 0:1024], AF.Sqrt, bias=epsb[:, 0:1]
                )
                nc.vector.reciprocal(rstdb[:, 0:1024], rstdb[:, 0:1024])  # rstd
                g_sb = [stile(dt=BF16) for _ in range(2)]
                for t in range(2):
                    xh = stile()
                    nc.vector.tensor_sub(xh[:, 0:L], xv[t], mub[:, 0:L])
                    nc.vector.tensor_mul(xh[:, 0:L], xh[:, 0:L], rstdb[:, 0:L])
                    nc.scalar.activation(
                        xh[:, 0:L],
                        xh[:, 0:L],
                        AF.Identity,
                        bias=nb_sb[:, t : t + 1],
                        scale=nw_sb[:, t : t + 1],
                    )
                    rowm = sp.tile([128, 1], F32, tag="rowm", name=_nm("rm"))
                    nc.vector.reduce_sum(rowm[:], xh[:, 0:L], axis=AX.X)
                    nc.vector.tensor_scalar_mul(rowm[:], rowm[:], 1.0 / L)
                    nc.vector.tensor_scalar_sub(xh[:, 0:L], xh[:, 0:L], rowm[:, 0:1])
                    nc.scalar.activation(g_sb[t][:, 0:L], xh[:, 0:L], AF.Gelu)

                # head: out[c] = sum_{t,p,l} g[t][p,l] * pw[t][p, c, l] + pb
                hsum = sp.tile([128, 8], F32, tag="hsum", name=_nm("hs"))
                for t in range(2):
                    for c in range(3):
                        hscr = stile()
                        nc.vector.tensor_mul(
                            hscr[:, 0:L],
                            g_sb[t][:, 0:L],
                            pw_sb[t][:, c * L : (c + 1) * L],
                        )
                        nc.vector.reduce_sum(
                            hsum[:, t * 3 + c : t * 3 + c + 1],
                            hscr[:, 0:L],
                            axis=AX.X,
                        )
                psh = ps2p.tile([1, 6], F32, tag="mm512", name=_nm("ph"))
                nc.tensor.matmul(
                    psh[:], ones2_sb[:], hsum[:, 0:6], start=True, stop=True
                )
                h6 = sp.tile([1, 6], F32, tag="h6", name=_nm("h6"))
                nc.vector.tensor_copy(h6[:], psh[:1, 0:6])
                o3 = sp.tile([1, 3], F32, tag="o3", name=_nm("o3"))
                nc.vector.tensor_add(o3[:], h6[:1, 0:3], h6[:1, 3:6])
                nc.vector.tensor_add(o3[:], o3[:], pb_sb[:])
                nc.sync.dma_start(out=out[b : b + 1, :], in_=o3[:])

            # Rolling window of 2 programs, offset by one segment so the
            # DVE-heavy segment (topk+gather) of one program overlaps the
            # PE-heavy segment (QKV/F or FFN) of the other; a finished
            # program is immediately replaced by the next batch.
            progs = [batch_program(b) for b in range(BL)]
            nxt = 0
            active = []

            def _admit():
                nonlocal nxt
                if nxt < BL:
                    active.append(progs[nxt])
                    nxt += 1
                    return True
                return False

            _admit()
            next(active[0])  # phase offset
            _admit()
            while active:
                for g_ in list(active):
                    try:
                        next(g_)
                    except StopIteration:
                        active.remove(g_)
                        _admit()

    _split_control_waits(nc)
    return nc


# ---------------------------------------------------------------- host side
_CACHE = {}


def _get_nc():
    if "nc" not in _CACHE:
        _CACHE["nc"] = build_nc()
    return _CACHE["nc"]


def kernel(**inputs):
    x_enc = np.asarray(inputs["x_enc"], dtype=np.float32)  # (B, L, C_IN)
    token_w = np.asarray(inputs["token_w"], dtype=np.float32)
    qw = np.asarray(inputs["qw"], dtype=np.float32)
    kw = np.asarray(inputs["kw"], dtype=np.float32)
    vw = np.asarray(inputs["vw"], dtype=np.float32)
    ow = np.asarray(inputs["ow"], dtype=np.float32)
    c1w = np.asarray(inputs["c1w"], dtype=np.float32)
    c2w = np.asarray(inputs["c2w"], dtype=np.float32)
    norm_w = np.asarray(inputs["norm_w"], dtype=np.float32)
    norm_b = np.asarray(inputs["norm_b"], dtype=np.float32)
    proj_w = np.asarray(inputs["proj_w"], dtype=np.float32)
    proj_b = np.asarray(inputs["proj_b"], dtype=np.float32)

    # host-side layout marshalling (no arithmetic)
    tokw = np.ascontiguousarray(token_w.transpose(1, 2, 0).reshape(63, D))
    # xemb[b, c*3+j, l] = x_enc[b, (l+j-1) % L, c]
    xt = x_enc.transpose(0, 2, 1)  # (B, C, L)
    xemb = np.ascontiguousarray(
        np.stack([np.roll(xt, 1 - j, axis=2) for j in range(3)], axis=2).reshape(
            B, 63, L
        )
    )
    shared = {
        "tokw": tokw,
        "wq": np.ascontiguousarray(qw.transpose(0, 2, 1)),
        "wk": np.ascontiguousarray(kw.transpose(0, 2, 1)),
        "wv": np.ascontiguousarray(vw.transpose(0, 2, 1)),
        "wo": np.ascontiguousarray(ow.transpose(0, 2, 1)),
        "wc1": np.ascontiguousarray(c1w.transpose(0, 2, 1)),
        "wc2": np.ascontiguousarray(c2w.transpose(0, 2, 1)),
        "nw": norm_w.reshape(D, 1).copy(),
        "nb": norm_b.reshape(D, 1).copy(),
        "pw": np.ascontiguousarray(
            proj_w.reshape(3, L, D).transpose(2, 0, 1)
        ).astype(ml_dtypes.bfloat16),
        "pb": proj_b.reshape(1, 3).copy(),
        "onescol": np.ones((128, 1), np.float32),
        "onescolf": np.ones((128, 1), np.float32),
        "onesrow": np.ones((1, 128), np.float32),
        "ones2d": np.ones((128, 128), np.float32),
        "negfifth": (-0.2 * np.eye(128)).astype(np.float32),
        "identr": np.eye(128, dtype=np.float32),
        "ident": np.eye(128, dtype=np.float32),
    }
    in_maps = []
    for core in range(NCORES):
        m = dict(shared)
        m["xemb"] = np.ascontiguousarray(xemb[core * BL : (core + 1) * BL])
        in_maps.append(m)

    nc = _get_nc()
    res_ = run_bass_kernel_spmd(nc, in_maps, core_ids=list(range(NCORES)))
    out = np.concatenate([res_.results[i]["out"] for i in range(NCORES)], axis=0)
    return out.astype(np.float32)


if __name__ == "__main__":
    import reference

    inputs = reference.setup_inputs()
    got = kernel(**{k: np.asarray(v) for k, v in inputs.items()})
    exp = np.asarray(reference.reference(**inputs))
    rel = np.abs(got - exp).max() / np.abs(exp).max()
    print("Relative error:", rel)



# revision 18
# speedup vs baseline: 1.2978x; 1.0410x over previous
"""Autoformer encoder (B=32, L=1024, D=256, 3 layers) on 8 TRN2 NeuronCores.

Data-parallel over batch (4 batches/core). All matmuls in f32r (full PE
rate, ~1.6e-4 rel err), fp32 residual stream and vector path.

AutoCorrelation without FFT: the lag-correlation
    C[tau] = (1/D) sum_l <q[:, l+tau], k[:, l]>
is computed as F[p, u] = sum_i sum_d k[d, 128i+p] * q2[d, 128i+u]
(PSUM-accumulated matmuls, q2 time-doubled), so that
C[tau] = sum_p F[p, p+tau]. The 128-row shear is done by bouncing F
through DRAM with row stride 1153 and reading back the strided view
[[1154, 128], [1, 1024]]; the partition sum is a ones-vector matmul.
Top-6 lags via vector.max/max_index.

The delay-rolled weighted sum of V uses register-dynamic slices into a
time-doubled V buffer. Each dynamic-AP instruction permanently consumes
~2 registers of the executing engine's 49 (no caching in this build), so
the 72 total gather slots are spread: 23 on ACT (scaled copy), 23 on DVE
(scalar_tensor_tensor FMA), 22 on Pool (FMA), 4 on PE (scaled-identity
matmul with dynamic rhs, PSUM-accumulated).
"""

import contextlib
import numpy as np
import ml_dtypes

import concourse.bass as bass
import concourse.mybir as mybir
from concourse import tile
from concourse.tile import TileContext
from concourse.tile_rust import add_dep_helper
from concourse.vector_clock import ScopedClock
from concourse.bass_utils import run_bass_kernel_spmd

F32 = mybir.dt.float32
F32R = mybir.dt.float32r
BF16 = mybir.dt.bfloat16
U32 = mybir.dt.uint32
AF = mybir.ActivationFunctionType
AX = mybir.AxisListType
ALU = mybir.AluOpType
ET = mybir.EngineType

B, L, C_IN = 32, 1024, 21
D, DFF, NL = 256, 1024, 3
TOPK = 6
NCORES = 8
BL = B // NCORES  # batches per core

HW = 1153  # F bounce row stride (1152 data + 1 pad)
FSH_SZ = 127 * HW + 1152


# ---------------------------------------------------------------- walrus fix
def _patched_drain_and_barrier(self, tick_clock, wait_clock):
    nc = self.nc
    drain_inst = nc.sync.drain()
    wait_clock.add_sem_waits(
        drain_inst.ins, ScopedClock({None: tick_clock.global_clock})
    )
    si = drain_inst.ins.sync_info
    if si is not None and len(si.on_wait) > 1:
        extra = list(si.on_wait[1:])
        del si.on_wait[1:]
        for w in extra:
            n = nc.sync.nop()
            n.ins.sync_info = mybir.SyncInfo(on_update=[], on_wait=[w])
    nc.all_engine_barrier()
    assert self.sems is not None
    popped = nc._tile_sem_poison_stack.pop()
    assert popped is self._sem_poison
    nc.clear_and_free_semaphores(list(self.sems.allocated().values()))
    nc.all_engine_barrier()


tile.TileContext._drain_and_barrier = _patched_drain_and_barrier

_wsctr = [0]


def _split_control_waits(nc):
    """This walrus build allows only ONE sync wait per instruction;
    hoist extras onto NoOps just before, same engine."""
    for fn in nc.m.functions:
        for bb in fn.blocks:
            out = []
            changed = False
            for inst in bb.instructions:
                si = getattr(inst, "sync_info", None)
                if si is not None and len(si.on_wait) > 1:
                    extra = list(si.on_wait[1:])
                    del si.on_wait[1:]
                    for w in extra:
                        _wsctr[0] += 1
                        n = mybir.InstNoOp(
                            name=f"I-waitsplit-{_wsctr[0]}", ins=[], outs=[]
                        )
                        n.engine = inst.engine
                        n.sync_info = mybir.SyncInfo(on_update=[], on_wait=[w])
                        out.append(n)
                        changed = True
                out.append(inst)
            if changed:
                bb.instructions[:] = out


def r(ap):
    return ap


def dep(a, b):
    add_dep_helper(a.ins, b.ins, sync=False, reason="gather order")


# ---------------------------------------------------------------- builder
def build_nc():
    nc = bass.Bass()
    P = lambda name, shape, dt=F32: nc.declare_dram_parameter(
        name, shape, dt, isOutput=False
    )
    xemb = P("xemb", [BL, 63, L], F32R)  # host im2col of token conv input
    tokw = P("tokw", [63, D], F32R)  # lhsT for token conv
    wq = P("wq", [NL, D, D], F32R)  # lhsT (= W.T) per layer
    wk = P("wk", [NL, D, D], F32R)
    wv = P("wv", [NL, D, D], F32R)
    wo = P("wo", [NL, D, D], F32R)
    wc1 = P("wc1", [NL, D, DFF], F32R)  # lhsT
    wc2 = P("wc2", [NL, DFF, D], F32R)  # lhsT
    nwp = P("nw", [D, 1])
    nbp = P("nb", [D, 1])
    pw = P("pw", [D, 3, L], BF16)  # proj_w as [d, class, l], bf16
    pb = P("pb", [1, 3])
    onescol = P("onescol", [128, 1], F32R)
    onescolf = P("onescolf", [128, 1])
    onesrow = P("onesrow", [1, 128])
    ones2d = P("ones2d", [128, 128], F32R)
    negfifth = P("negfifth", [128, 128], F32R)  # -0.2 * I, lhsT for decomp
    identr = P("identr", [128, 128], F32R)  # I, lhsT for decomp
    ident = P("ident", [128, 128])
    out = nc.declare_dram_parameter("out", [BL, 3], F32, isOutput=True)
    import os
    KDBG = bool(os.environ.get("KDBG"))
    dbg = {}
    if KDBG:
        for nm_, shp, dt_ in [
            ("dbg_x0", [2, 128, L], F32),
            ("dbg_k", [128, L], F32),
            ("dbg_q2", [128, 4096], F32),
            ("dbg_f", [128, 1152], F32),
            ("dbg_h", [128, L], F32),
            ("dbg_c", [1, L], F32),
            ("dbg_ix", [1, 8], U32),
            ("dbg_wb", [128, 8], F32),
            ("dbg_a", [128, 2048], F32),
            ("dbg_s", [2, 128, L + 4], F32),
            ("dbg_x1", [2, 128, L + 4], F32),
            ("dbg_xo", [2, 128, L], F32),
        ]:
            dbg[nm_] = nc.declare_dram_parameter(nm_, shp, dt_, isOutput=True)

    fsh = nc.dram_tensor("fsh", [BL * NL, FSH_SZ], F32R)

    with TileContext(nc) as tc:
        ctx = contextlib.ExitStack()
        with ctx:
            wp = ctx.enter_context(tc.tile_pool(name="weights", bufs=1))
            res = ctx.enter_context(tc.tile_pool(name="res", bufs=8))
            scr = ctx.enter_context(tc.tile_pool(name="scr", bufs=8))
            big = ctx.enter_context(tc.tile_pool(name="big4k", bufs=3))
            gat = ctx.enter_context(tc.tile_pool(name="gat", bufs=3))
            sp = ctx.enter_context(tc.tile_pool(name="small", bufs=4))
            ps = ctx.enter_context(tc.tile_pool(name="psum", bufs=3, space="PSUM"))
            ps2p = ctx.enter_context(
                tc.tile_pool(name="psumB", bufs=2, space="PSUM")
            )

            _names = [0]

            def _nm(pfx):
                _names[0] += 1
                return f"{pfx}{_names[0]}"

            def rtile():
                return res.tile([128, L + 4], F32R, tag="res", name=_nm("rt"))

            def stile(fr=1152, dt=F32, p=128):
                return scr.tile([p, fr], dt, tag="scr", name=_nm("st"))

            # ---- load weights to SBUF once
            tokw_sb = wp.tile([63, D], F32R, tag="tokw")
            nc.sync.dma_start(out=tokw_sb[:], in_=tokw[:])
            ones_sb = wp.tile([128, 1], F32R, tag="ones")
            ones2_sb = wp.tile([128, 1], F32, tag="ones2")
            nc.sync.dma_start(out=ones_sb[:], in_=onescol[:])
            nc.sync.dma_start(out=ones2_sb[:], in_=onescolf[:])
            onesr_sb = wp.tile([1, 128], F32, tag="onesr")
            nc.sync.dma_start(out=onesr_sb[:], in_=onesrow[:])
            ones2d_sb = wp.tile([128, 128], F32R, tag="ones2d")
            nc.sync.dma_start(out=ones2d_sb[:], in_=ones2d[:])
            negf_sb = wp.tile([128, 128], F32R, tag="negf")
            nc.sync.dma_start(out=negf_sb[:], in_=negfifth[:])
            idr_sb = wp.tile([128, 128], F32R, tag="idr")
            nc.sync.dma_start(out=idr_sb[:], in_=identr[:])
            id_sb = wp.tile([128, 128], F32, tag="id")
            nc.sync.dma_start(out=id_sb[:], in_=ident[:])
            nw_sb = wp.tile([128, 2], F32, tag="nw")  # col t = tile t
            nb_sb = wp.tile([128, 2], F32, tag="nb")
            for t in range(2):
                nc.sync.dma_start(
                    out=nw_sb[:, t : t + 1], in_=nwp[t * 128 : (t + 1) * 128, :]
                )
                nc.sync.dma_start(
                    out=nb_sb[:, t : t + 1], in_=nbp[t * 128 : (t + 1) * 128, :]
                )
            pb_sb = wp.tile([1, 3], F32, tag="pb")
            nc.sync.dma_start(out=pb_sb[:], in_=pb[:])

            # layer weights streamed per (b, l), double-buffered
            ws = ctx.enter_context(tc.tile_pool(name="wstream", bufs=2))

            def lload(name, src, l, kt, cols):
                tl = ws.tile(
                    [128, cols], F32R, tag=f"{name}k{kt}", name=_nm(f"{name}{l}")
                )
                nc.sync.dma_start(out=tl[:], in_=src[l, kt * 128 : (kt + 1) * 128, :])
                return tl
            pw_sb = [None, None]
            for t in range(2):
                pw_sb[t] = wp.tile([128, 3 * L], BF16, tag=f"pw{t}", name=f"pw{t}")
                nc.sync.dma_start(
                    out=pw_sb[t][:].rearrange("p (c l) -> p c l", c=3),
                    in_=pw[t * 128 : (t + 1) * 128, :, :],
                )

            # persistent per-engine delay registers + snapped values
            engs = {
                "ACT": nc.engines[ET.Activation],
                "DVE": nc.engines[ET.DVE],
                "POOL": nc.engines[ET.Pool],
                "PE": nc.engines[ET.PE],
            }
            dreg = {k: e.alloc_register(f"dly_{k}") for k, e in engs.items()}
            dval = {
                k: nc.snap(rg, donate=True, min_val=0, max_val=1023)
                for k, rg in dreg.items()
            }

            def proj(dst_fn, w_sb_l, src_aps):
                """dst[mt][chunk] <- sum_kt w[kt].T @ src[kt][:, chunk]."""
                for mt in range(2):
                    for ch in range(2):
                        p5 = ps2p.tile([128, 512], F32, tag="mm512", name=_nm("p5"))
                        for kt in range(2):
                            nc.tensor.matmul(
                                p5[:],
                                r(w_sb_l[kt][:, mt * 128 : (mt + 1) * 128]),
                                r(src_aps[kt][:, ch * 512 : (ch + 1) * 512]),
                                start=(kt == 0),
                                stop=(kt == 1),
                            )
                        dst_fn(mt, ch, p5)

            def batch_program(b):
                # ---- token embedding: x[d, l], 2 tiles, data in [0, L)
                xe_sb = stile(fr=L, p=63, dt=F32R)
                nc.sync.dma_start(out=xe_sb[:], in_=xemb[b, :, :])
                x_sb = [rtile() for _ in range(2)]
                for mt in range(2):
                    for ch in range(2):
                        p5 = ps2p.tile([128, 512], F32, tag="mm512", name=_nm("p5"))
                        nc.tensor.matmul(
                            p5[:],
                            r(tokw_sb[:, mt * 128 : (mt + 1) * 128]),
                            r(xe_sb[:, ch * 512 : (ch + 1) * 512]),
                            start=True,
                            stop=True,
                        )
                        nc.vector.tensor_copy(
                            x_sb[mt][:, ch * 512 : (ch + 1) * 512], p5[:]
                        )

                if KDBG and b == 0:
                    for t in range(2):
                        nc.sync.dma_start(
                            out=dbg["dbg_x0"][t], in_=x_sb[t][:, 0:L].bitcast(F32)
                        )

                for l in range(NL):
                    last_bl = (b == BL - 1) and (l == NL - 1)
                    tap = KDBG and b == 0 and l == 0
                    wq_l = [lload("wq", wq, l, t, D) for t in range(2)]
                    wk_l = [lload("wk", wk, l, t, D) for t in range(2)]
                    wv_l = [lload("wv", wv, l, t, D) for t in range(2)]
                    wo_l = [lload("wo", wo, l, t, D) for t in range(2)]
                    wc1_l = [lload("wc1", wc1, l, t, DFF) for t in range(2)]
                    wc2_l = [lload("wc2", wc2, l, t, D) for t in range(8)]
                    # ---- Q (doubled, stacked kt: col 2048*kt + u), K, V (same)
                    q2_sb = big.tile([128, 4096], F32R, tag="big4k", name=_nm("q2"))
                    v4_sb = big.tile([128, 4096], F32R, tag="big4k", name=_nm("v4"))
                    k_sb = [stile(dt=F32R) for _ in range(2)]

                    def dbl_out(dst):
                        def f(mt, ch, p5):
                            base = 2048 * mt + ch * 512
                            nc.vector.tensor_copy(dst[:, base : base + 512], p5[:])
                            nc.scalar.copy(dst[:, base + 1024 : base + 1536], p5[:])

                        return f

                    def k_out(mt, ch, p5):
                        nc.scalar.copy(
                            k_sb[mt][:, ch * 512 : (ch + 1) * 512], p5[:]
                        )

                    xin = [x_sb[t][:, 0:L] for t in range(2)]
                    proj(dbl_out(q2_sb), wq_l, xin)
                    proj(k_out, wk_l, xin)
                    proj(dbl_out(v4_sb), wv_l, xin)

                    if tap:
                        nc.sync.dma_start(
                            out=dbg["dbg_k"][:], in_=k_sb[0][:, 0:L].bitcast(F32)
                        )
                        nc.sync.dma_start(
                            out=dbg["dbg_q2"][:], in_=q2_sb[:].bitcast(F32)
                        )

                    # ---- F[p, u] = sum_i sum_d k[d,128i+p] q2[d,128i+u]
                    # F in two PSUM tiles so "big" slots stay 2 banks and F
                    # can overlap the FFN's ps2 accumulators. Each 384-wide
                    # chunk is bank-aligned (a matmul output may not cross a
                    # 512-f32 PSUM bank).
                    fps_a = ps.tile([128, 1024], F32, tag="big", name=_nm("fpsa"))
                    fps_b = ps2p.tile([128, 512], F32, tag="mm512", name=_nm("fpsb"))
                    for ch in range(3):  # 3 x 384
                        dstp = (
                            fps_a[:, ch * 512 : ch * 512 + 384]
                            if ch < 2
                            else fps_b[:, 0:384]
                        )
                        for i in range(8):
                            for kt in range(2):
                                base = 2048 * kt + i * 128 + ch * 384
                                nc.tensor.matmul(
                                    dstp,
                                    r(k_sb[kt][:, i * 128 : (i + 1) * 128]),
                                    r(q2_sb[:, base : base + 384]),
                                    start=((i, kt) == (0, 0)),
                                    stop=((i, kt) == (7, 1)),
                                )
                    # bounce through DRAM with the shear stride
                    f_sb = stile(dt=F32R)
                    nc.vector.tensor_copy(
                        f_sb[:, 0:768].rearrange("p (c u) -> p c u", c=2),
                        fps_a[:].rearrange("p (c u) -> p c u", c=2)[:, :, 0:384],
                    )
                    nc.vector.tensor_copy(f_sb[:, 768:1152], fps_b[:, 0:384])
                    frow = fsh[b * NL + l, :]
                    wview = bass.AP(frow.tensor, frow.offset, [[HW, 128], [1, 1152]])
                    fwr = nc.sync.dma_start(out=wview, in_=f_sb[:, 0:1152])
                    hview = bass.AP(
                        frow.tensor, frow.offset, [[HW + 1, 128], [1, 1024]]
                    )
                    h_sb = stile(dt=F32R)
                    hrd = nc.sync.dma_start(out=h_sb[:, 0:1024], in_=hview)
                    add_dep_helper(
                        hrd.ins, fwr.ins, sync=True, reason="hankel read after write"
                    )
                    yield
                    if tap:
                        nc.sync.dma_start(
                            out=dbg["dbg_f"][:], in_=f_sb[:, 0:1152].bitcast(F32)
                        )
                        nc.sync.dma_start(
                            out=dbg["dbg_h"][:], in_=h_sb[:, 0:1024].bitcast(F32)
                        )

                    # ---- C[tau] = (1/256) * sum_p H[p, tau]; top-6; softmax.
                    # All-ones lhsT broadcasts the partition sum to all 128
                    # partitions, so the whole softmax chain runs redundantly
                    # per-partition (same modeled cost: free-size only) and
                    # no PE/PSUM broadcast of the weights is needed.
                    c_sb = stile()
                    for ch in range(2):
                        cp = ps2p.tile([128, 512], F32, tag="mm512", name=_nm("cp"))
                        nc.tensor.matmul(
                            cp[:],
                            r(ones2d_sb[:]),
                            r(h_sb[:, ch * 512 : (ch + 1) * 512]),
                            start=True,
                            stop=True,
                        )
                        nc.scalar.activation(
                            c_sb[:, ch * 512 : (ch + 1) * 512],
                            cp[:],
                            AF.Copy,
                            scale=1.0 / D,
                        )
                    mx = sp.tile([128, 8], F32, tag="mx", name=_nm("mx"))
                    ix = sp.tile([128, 8], U32, tag="ix", name=_nm("ix"))
                    nc.vector.max(out=mx[:], in_=c_sb[:, 0:1024])
                    nc.vector.max_index(
                        out=ix[:], in_max=mx[:], in_values=c_sb[:, 0:1024]
                    )
                    negmax = sp.tile([128, 1], F32, tag="negmax", name=_nm("ng"))
                    nc.vector.tensor_scalar_mul(negmax[:], mx[:, 0:1], -1.0)
                    ex = sp.tile([128, 8], F32, tag="ex", name=_nm("ex"))
                    nc.scalar.activation(
                        ex[:, 0:TOPK], mx[:, 0:TOPK], AF.Exp, bias=negmax[:, 0:1]
                    )
                    esum = sp.tile([128, 1], F32, tag="esum", name=_nm("es"))
                    nc.vector.reduce_sum(esum[:], ex[:, 0:TOPK], axis=AX.X)
                    rinv = sp.tile([128, 1], F32, tag="rinv", name=_nm("ri"))
                    nc.vector.reciprocal(rinv[:], esum[:])
                    wb = sp.tile([128, 8], F32, tag="wb", name=_nm("wb"))
                    nc.vector.tensor_scalar_mul(
                        wb[:, 0:TOPK], ex[:, 0:TOPK], rinv[:, 0:1]
                    )
                    if tap:
                        nc.sync.dma_start(out=dbg["dbg_c"][:], in_=c_sb[:1, 0:L])
                        nc.sync.dma_start(out=dbg["dbg_ix"][:], in_=ix[:1])
                        nc.sync.dma_start(
                            out=dbg["dbg_wb"][:, 0:TOPK], in_=wb[:, 0:TOPK]
                        )

                    # ---- a[:, 1024*t + u] = sum_i w_i V[t][:, (u+d_i) % L]
                    a_sb = gat.tile([128, 2048], F32R, tag="gat", name=_nm("a"))
                    tq_sb = gat.tile([128, 2048], F32R, tag="gat", name=_nm("tq"))
                    pq_sb = gat.tile([128, 2048], F32R, tag="gat", name=_nm("pq"))
                    v4r = v4_sb[:].rearrange("p (b u) -> p b u", b=2)
                    a3 = a_sb[:].rearrange("p (b u) -> p b u", b=2)
                    tq3 = tq_sb[:].rearrange("p (b u) -> p b u", b=2)
                    pq3 = pq_sb[:].rearrange("p (b u) -> p b u", b=2)

                    def ld(ekey, i):
                        return engs[ekey].reg_load(dreg[ekey], ix[:1, i : i + 1])

                    def act_copy(i, dst3):
                        return nc.scalar.activation(
                            dst3,
                            v4r[:, :, bass.ds(dval["ACT"], 1024)],
                            AF.Copy,
                            scale=wb[:, i : i + 1],
                        )

                    def fma(ekey, i):
                        eng = nc.vector if ekey == "DVE" else nc.gpsimd
                        return eng.scalar_tensor_tensor(
                            a3,
                            v4r[:, :, bass.ds(dval[ekey], 1024)],
                            wb[:, i : i + 1],
                            a3,
                            op0=ALU.mult,
                            op1=ALU.add,
                        )

                    if not last_bl:
                        l0 = ld("ACT", 0)
                        o0 = act_copy(0, a3)
                        dep(o0, l0)
                        l1 = ld("ACT", 1)
                        dep(l1, o0)
                        o1 = act_copy(1, tq3)
                        dep(o1, l1)
                        l2 = ld("DVE", 2)
                        o2 = fma("DVE", 2)
                        dep(o2, l2)
                        l3 = ld("DVE", 3)
                        dep(l3, o2)
                        o3_ = fma("DVE", 3)
                        dep(o3_, l3)
                        # Pool: tensor_tensor mult with broadcast weight
                        l4 = ld("POOL", 4)
                        o4 = nc.gpsimd.tensor_mul(
                            pq3,
                            v4r[:, :, bass.ds(dval["POOL"], 1024)],
                            wb[:, 4:5].to_broadcast([128, 2, 1024]),
                        )
                        dep(o4, l4)
                        ad4 = nc.vector.tensor_add(a_sb[:], a_sb[:], pq_sb[:])
                        l5 = ld("POOL", 5)
                        dep(l5, o4)
                        o5 = nc.gpsimd.tensor_mul(
                            pq3,
                            v4r[:, :, bass.ds(dval["POOL"], 1024)],
                            wb[:, 5:6].to_broadcast([128, 2, 1024]),
                        )
                        dep(o5, l5)
                        nc.vector.tensor_add(a_sb[:], a_sb[:], pq_sb[:])
                        nc.vector.tensor_add(a_sb[:], a_sb[:], tq_sb[:])
                    else:
                        # last (b, l): ACT slot 0, DVE slot 1, PE slots 2..5
                        l0 = ld("ACT", 0)
                        o0 = act_copy(0, a3)
                        dep(o0, l0)
                        l1 = ld("DVE", 1)
                        o1 = fma("DVE", 1)
                        dep(o1, l1)
                        pe = engs["PE"]
                        wds = []
                        for i in range(2, 6):
                            wd = stile(fr=128, dt=F32R)
                            nc.vector.tensor_scalar(
                                wd[:, 0:128],
                                id_sb[:],
                                wb[:, i : i + 1],
                                None,
                                op0=ALU.mult,
                            )
                            wds.append(wd)
                        pgs = []
                        prev = None
                        for t in range(2):
                            for c in range(2):
                                pg = ps2p.tile(
                                    [128, 512], F32, tag="mm512", name=_nm("pg")
                                )
                                for ii, i in enumerate(range(2, 6)):
                                    lp = pe.reg_load(dreg["PE"], ix[:1, i : i + 1])
                                    if prev is not None:
                                        dep(lp, prev)
                                    al = pe.reg_alu(
                                        dreg["PE"],
                                        dreg["PE"],
                                        2048 * t + 512 * c,
                                        ALU.add,
                                    )
                                    dep(al, lp)
                                    mm = nc.tensor.matmul(
                                        pg[:],
                                        r(wds[ii][:, 0:128]),
                                        r(v4_sb[:, bass.ds(dval["PE"], 512)]),
                                        start=(ii == 0),
                                        stop=(ii == 3),
                                    )
                                    dep(mm, al)
                                    prev = mm
                                pgs.append((t, c, pg))
                        for t, c, pg in pgs:
                            base = 1024 * t + 512 * c
                            nc.vector.tensor_add(
                                a_sb[:, base : base + 512],
                                a_sb[:, base : base + 512],
                                pg[:],
                            )

                    if tap:
                        nc.sync.dma_start(
                            out=dbg["dbg_a"][:], in_=a_sb[:].bitcast(F32)
                        )

                    yield

                    # ---- O-projection; s = x + a into padded tile (data at 2)
                    s_sb = [rtile() for _ in range(2)]

                    def o_out(mt, ch, p5):
                        nc.vector.tensor_add(
                            s_sb[mt][:, 2 + ch * 512 : 2 + (ch + 1) * 512],
                            x_sb[mt][:, ch * 512 : (ch + 1) * 512],
                            p5[:],
                        )

                    proj(
                        o_out,
                        wo_l,
                        [a_sb[:, 1024 * t : 1024 * (t + 1)] for t in range(2)],
                    )

                    # ---- series_decomp on PE: dst[:, off+u] =
                    # src[2+u] - 0.2*sum_{j=0..4} src[j+u], as 6 PSUM-
                    # accumulated identity matmuls per 512-chunk. dst must
                    # be a different tile than src.
                    def decomp(src_pad, dst, dst_off):
                        # src_pad: [128, 1028] with data in cols [2, 1026)
                        nc.vector.tensor_copy(
                            src_pad[:, 0:2], src_pad[:, 2:3].to_broadcast([128, 2])
                        )
                        nc.vector.tensor_copy(
                            src_pad[:, 1026:1028],
                            src_pad[:, 1025:1026].to_broadcast([128, 2]),
                        )
                        idr = idr_sb[:]
                        for c in range(2):
                            pg = ps2p.tile([128, 512], F32, tag="mm512", name=_nm("dc"))
                            nc.tensor.matmul(
                                pg[:],
                                idr,
                                src_pad[:, 2 + c * 512 : 2 + c * 512 + 512],
                                start=True,
                                stop=False,
                            )
                            for j in range(5):
                                nc.tensor.matmul(
                                    pg[:],
                                    negf_sb[:],
                                    src_pad[:, j + c * 512 : j + c * 512 + 512],
                                    start=False,
                                    stop=(j == 4),
                                )
                            nc.scalar.activation(
                                dst[:, dst_off + c * 512 : dst_off + c * 512 + 512],
                                pg[:],
                                AF.Copy,
                            )

                    if tap:
                        for t in range(2):
                            nc.sync.dma_start(
                                out=dbg["dbg_s"][t], in_=s_sb[t][:].bitcast(F32)
                            )

                    # x1 = decomp(s) into x_sb (x is dead once s is formed)
                    for t in range(2):
                        decomp(s_sb[t], x_sb[t], dst_off=2)
                    x1_sb = x_sb
                    if tap:
                        for t in range(2):
                            nc.sync.dma_start(
                                out=dbg["dbg_x1"][t], in_=x1_sb[t][:].bitcast(F32)
                            )

                    # ---- FFN: y = gelu(c1 @ x1); s2 = x1 + c2 @ y (in place)
                    x1v = [x1_sb[t][:, 2:1026] for t in range(2)]
                    ps2 = [
                        ps.tile([128, 1024], F32, tag="big", name=_nm("ps2"))
                        for _ in range(2)
                    ]
                    for ft in range(8):
                        y_sb = stile(dt=F32R)
                        for ch in range(2):
                            p5 = ps2p.tile([128, 512], F32, tag="mm512", name=_nm("p5"))
                            for kt in range(2):
                                nc.tensor.matmul(
                                    p5[:],
                                    r(wc1_l[kt][:, ft * 128 : (ft + 1) * 128]),
                                    r(x1v[kt][:, ch * 512 : (ch + 1) * 512]),
                                    start=(kt == 0),
                                    stop=(kt == 1),
                                )
                            nc.scalar.activation(
                                y_sb[:, ch * 512 : (ch + 1) * 512], p5[:], AF.Gelu
                            )
                        for mt in range(2):
                            for ch in range(2):
                                nc.tensor.matmul(
                                    ps2[mt][:, ch * 512 : (ch + 1) * 512],
                                    r(wc2_l[ft][:, mt * 128 : (mt + 1) * 128]),
                                    r(y_sb[:, ch * 512 : (ch + 1) * 512]),
                                    start=(ft == 0),
                                    stop=(ft == 7),
                                )
                    # s2 = x1 + ffn(x1) into s_sb (s is dead once x1 exists)
                    for mt in range(2):
                        for ch in range(2):
                            nc.vector.tensor_add(
                                s_sb[mt][:, 2 + ch * 512 : 2 + (ch + 1) * 512],
                                x1v[mt][:, ch * 512 : (ch + 1) * 512],
                                ps2[mt][:, ch * 512 : (ch + 1) * 512],
                            )
                    for t in range(2):
                        decomp(s_sb[t], x_sb[t], dst_off=0)
                    yield
                    if tap:
                        for t in range(2):
                            nc.sync.dma_start(
                                out=dbg["dbg_xo"][t], in_=x_sb[t][:, 0:L].bitcast(F32)
                            )

                # ---- my_layernorm + gelu + head.
                # xh - mean_l(xh) = nw * (z - mean_l(z)) with
                # z = (x - mu) * rstd  (norm_b cancels in the subtraction).
                xv = [x_sb[t][:, 0:L] for t in range(2)]
                xsq = [stile(dt=F32R) for _ in range(2)]
                for t in range(2):
                    nc.scalar.activation(xsq[t][:, 0:L], xv[t], AF.Square)
                mub = stile()
                mu2b = stile()
                rstdb = stile()
                for ch in range(2):
                    cs = ps2p.tile([128, 512], F32, tag="mm512", name=_nm("cs"))
                    for kt in range(2):
                        nc.tensor.matmul(
                            cs[:],
                            r(ones2d_sb[:]),
                            r(xv[kt][:, ch * 512 : (ch + 1) * 512]),
                            start=(kt == 0),
                            stop=(kt == 1),
                        )
                    nc.scalar.activation(
                        mub[:, ch * 512 : (ch + 1) * 512], cs[:], AF.Copy, scale=1.0 / D
                    )
                    nc.scalar.activation(
                        mu2b[:, ch * 512 : (ch + 1) * 512],
                        cs[:],
                        AF.Square,
                        scale=1.0 / D,
                    )
                    cq = ps2p.tile([128, 512], F32, tag="mm512", name=_nm("cq"))
                    for kt in range(2):
                        nc.tensor.matmul(
                            cq[:],
                            r(ones2d_sb[:]),
                            r(xsq[kt][:, ch * 512 : (ch + 1) * 512]),
                            start=(kt == 0),
                            stop=(kt == 1),
                        )
                    nc.scalar.activation(
                        rstdb[:, ch * 512 : (ch + 1) * 512],
                        cq[:],
                        AF.Copy,
                        scale=1.0 / D,
                    )
                yield
                epsb = sp.tile([128, 1], F32, tag="epsb", name=_nm("ep"))
                nc.vector.memset(epsb[:], 1e-5)
                nc.vector.tensor_sub(
                    rstdb[:, 0:1024], rstdb[:, 0:1024], mu2b[:, 0:1024]
                )
                nc.scalar.activation(
                    rstdb[:, 0:1024], rstdb[:, 0:1024], AF.Sqrt, bias=epsb[:, 0:1]
                )
                nc.vector.reciprocal(rstdb[:, 0:1024], rstdb[:, 0:1024])
                g_sb = [stile(dt=BF16) for _ in range(2)]
                for t in range(2):
                    xh = stile()
                    nc.vector.tensor_sub(xh[:, 0:L], xv[t], mub[:, 0:L])
                    nc.vector.tensor_mul(xh[:, 0:L], xh[:, 0:L], rstdb[:, 0:L])
                    rowm = sp.tile([128, 1], F32, tag="rowm", name=_nm("rm"))
                    nc.vector.reduce_sum(rowm[:], xh[:, 0:L], axis=AX.X)
                    nc.vector.tensor_scalar_mul(rowm[:], rowm[:], 1.0 / L)
                    nc.vector.tensor_scalar(
                        xh[:, 0:L],
                        xh[:, 0:L],
                        rowm[:, 0:1],
                        nw_sb[:, t : t + 1],
                        op0=ALU.subtract,
                        op1=ALU.mult,
                    )
                    nc.scalar.activation(g_sb[t][:, 0:L], xh[:, 0:L], AF.Gelu)
                yield

                # head: out[c] = sum_{t,p,l} g[t][p,l] * pw[t][p, c, l] + pb
                # (muls on Pool, reduces on DVE)
                hsum = sp.tile([128, 8], F32, tag="hsum", name=_nm("hs"))
                for t in range(2):
                    for c in range(3):
                        hscr = stile()
                        nc.gpsimd.tensor_mul(
                            hscr[:, 0:L],
                            g_sb[t][:, 0:L],
                            pw_sb[t][:, c * L : (c + 1) * L],
                        )
                        nc.vector.reduce_sum(
                            hsum[:, t * 3 + c : t * 3 + c + 1],
                            hscr[:, 0:L],
                            axis=AX.X,
                        )
                # partition-sum of hsum on Pool (keeps the tail off the
                # PE queue so the next program's matmuls aren't blocked)
                h6 = sp.tile([128, 8], F32, tag="h6", name=_nm("h6"))
                from concourse import bass_isa
                nc.gpsimd.partition_all_reduce(
                    h6[:, 0:6], hsum[:, 0:6], channels=128,
                    reduce_op=bass_isa.ReduceOp.add,
                )
                o3 = sp.tile([1, 3], F32, tag="o3", name=_nm("o3"))
                nc.vector.tensor_add(o3[:], h6[:1, 0:3], h6[:1, 3:6])
                nc.vector.tensor_add(o3[:], o3[:], pb_sb[:])
                nc.sync.dma_start(out=out[b : b + 1, :], in_=o3[:])

            # Rolling window of 2 programs, offset by one segment so the
            # DVE-heavy segment (topk+gather) of one program overlaps the
            # PE-heavy segment (QKV/F or FFN) of the other; a finished
            # program is immediately replaced by the next batch.
            progs = [batch_program(b) for b in range(BL)]
            nxt = 0
            active = []

            def _admit():
                nonlocal nxt
                if nxt < BL:
                    active.append(progs[nxt])
                    nxt += 1
                    return True
                return False

            _admit()
            next(active[0])  # phase offset
            _admit()
            while active:
                for g_ in list(active):
                    try:
                        next(g_)
                    except StopIteration:
                        active.remove(g_)
                        _admit()

    _split_control_waits(nc)
    return nc


# ---------------------------------------------------------------- host side
_CACHE = {}


def _get_nc():
    if "nc" not in _CACHE:
        _CACHE["nc"] = build_nc()
    return _CACHE["nc"]


def kernel(**inputs):
    x_enc = np.asarray(inputs["x_enc"], dtype=np.float32)  # (B, L, C_IN)
    token_w = np.asarray(inputs["token_w"], dtype=np.float32)
    qw = np.asarray(inputs["qw"], dtype=np.float32)
    kw = np.asarray(inputs["kw"], dtype=np.float32)
    vw = np.asarray(inputs["vw"], dtype=np.float32)
    ow = np.asarray(inputs["ow"], dtype=np.float32)
    c1w = np.asarray(inputs["c1w"], dtype=np.float32)
    c2w = np.asarray(inputs["c2w"], dtype=np.float32)
    norm_w = np.asarray(inputs["norm_w"], dtype=np.float32)
    norm_b = np.asarray(inputs["norm_b"], dtype=np.float32)
    proj_w = np.asarray(inputs["proj_w"], dtype=np.float32)
    proj_b = np.asarray(inputs["proj_b"], dtype=np.float32)

    # host-side layout marshalling (no arithmetic)
    tokw = np.ascontiguousarray(token_w.transpose(1, 2, 0).reshape(63, D))
    # xemb[b, c*3+j, l] = x_enc[b, (l+j-1) % L, c]
    xt = x_enc.transpose(0, 2, 1)  # (B, C, L)
    xemb = np.ascontiguousarray(
        np.stack([np.roll(xt, 1 - j, axis=2) for j in range(3)], axis=2).reshape(
            B, 63, L
        )
    )
    shared = {
        "tokw": tokw,
        "wq": np.ascontiguousarray(qw.transpose(0, 2, 1)),
        "wk": np.ascontiguousarray(kw.transpose(0, 2, 1)),
        "wv": np.ascontiguousarray(vw.transpose(0, 2, 1)),
        "wo": np.ascontiguousarray(ow.transpose(0, 2, 1)),
        "wc1": np.ascontiguousarray(c1w.transpose(0, 2, 1)),
        "wc2": np.ascontiguousarray(c2w.transpose(0, 2, 1)),
        "nw": norm_w.reshape(D, 1).copy(),
        "nb": norm_b.reshape(D, 1).copy(),
        "pw": np.ascontiguousarray(
            proj_w.reshape(3, L, D).transpose(2, 0, 1)
        ).astype(ml_dtypes.bfloat16),
        "pb": proj_b.reshape(1, 3).copy(),
        "onescol": np.ones((128, 1), np.float32),
        "onescolf": np.ones((128, 1), np.float32),
        "onesrow": np.ones((1, 128), np.float32),
        "ones2d": np.ones((128, 128), np.float32),
        "negfifth": (-0.2 * np.eye(128)).astype(np.float32),
        "identr": np.eye(128, dtype=np.float32),
        "ident": np.eye(128, dtype=np.float32),
    }
    in_maps = []
    for core in range(NCORES):
        m = dict(shared)
        m["xemb"] = np.ascontiguousarray(xemb[core * BL : (core + 1) * BL])
        in_maps.append(m)

    nc = _get_nc()
    res_ = run_bass_kernel_spmd(nc, in_maps, core_ids=list(range(NCORES)))
    out = np.concatenate([res_.results[i]["out"] for i in range(NCORES)], axis=0)
    return out.astype(np.float32)


if __name__ == "__main__":
    import reference

    inputs = reference.setup_inputs()
    got = kernel(**{k: np.asarray(v) for k, v in inputs.items()})
    exp = np.asarray(reference.reference(**inputs))
    rel = np.abs(got - exp).max() / np.abs(exp).max()
    print("Relative error:", rel)

